# revision 1
# baseline (speedup 1.0000x reference)
"""GraphConv (DGL norm='both') + log_softmax on 8 Trainium2 NeuronCores.

Strategy (per sharding hint): partition nodes across the 8 cores by range.
  Launch A (per core): project its 12500-node slice m = (h @ W) * out_deg^-1/2.
  Host: concatenate the 8 projected shards into a replicated gather table.
  Launch B (per core): for its 12500 dst nodes, gather m[src] rows for all
  in-edges (dma_gather, edges pre-sorted by dst group), segment-sum via
  one-hot matmuls accumulating in PSUM, then norm/bias/log_softmax.

Degrees and the sorted/padded edge metadata are sharding-prep computed on the
host (numpy); all FLOPs on h/W/b/m (projection, normalization, aggregation,
softmax) run on device.
"""

import numpy as np
import ml_dtypes

import concourse.bass as bass
import concourse.bacc as bacc
import concourse.mybir as mybir
import concourse.tile as tile
from concourse.bass import AP
from concourse.bass_utils import run_bass_kernel_spmd

P = 128
N_NODES = 100000
N_EDGES = 3200000
IN_DIM = 256
OUT_DIM = 64
NCORES = 8
G = N_NODES // NCORES            # 12500 nodes per core
NG = (G + P - 1) // P            # 98 groups of 128 dst nodes (last has 84)
GPAD = NG * P                    # 12544
NT = 4                           # gather sub-tables (int16 index limit)
TROWS = (NCORES * GPAD) // NT    # 25088 rows per sub-table
ROUND_G = 4                      # dst groups per gather round
HBLK = 8                         # dst groups per hT load in launch A
PAD_LDST = 200.0                 # local-dst for padded edges (>127, exact bf16)

_f32 = mybir.dt.float32
_bf16 = mybir.dt.bfloat16
_i16 = mybir.dt.int16


def _expand_mid(ap, n):
    """[P, C] AP -> [P, n, C] AP repeating each partition row n times
    (middle broadcast keeps the last dim packed, so DVE 2x mode applies)."""
    (ps, pc), (cs, cc) = ap.ap[0], ap.ap[1]
    return AP(ap.tensor, ap.offset, [[ps, pc], [0, n], [cs, cc]])


import contextlib


# ---------------------------------------------------------------- launch A
def build_launch_a(repeat=1):
    nc = bacc.Bacc("TRN2", target_bir_lowering=False, debug=False,
                   num_devices=NCORES)
    hT = nc.dram_tensor("hT", [2, P, GPAD], _f32, kind="ExternalInput")
    W = nc.dram_tensor("W", [IN_DIM, OUT_DIM], _f32, kind="ExternalInput")
    odeg = nc.dram_tensor("odeg", [P, NG], _f32, kind="ExternalInput")
    m = nc.dram_tensor("m", [GPAD, OUT_DIM], _f32, kind="ExternalOutput")

    with tile.TileContext(nc) as tc:
        loop = tc.For_i(0, repeat, 1) if repeat > 1 \
            else contextlib.nullcontext()
        with loop, \
                tc.tile_pool(name="const", bufs=1) as cpool, \
                tc.tile_pool(name="hblk", bufs=3) as hpool, \
                tc.tile_pool(name="work", bufs=4) as pool, \
                tc.tile_pool(name="psum", bufs=4, space="PSUM") as psum:
            w0 = cpool.tile([P, OUT_DIM], _f32, tag="w0")
            w1 = cpool.tile([P, OUT_DIM], _f32, tag="w1")
            nc.sync.dma_start(out=w0[:], in_=W[0:P, :])
            nc.sync.dma_start(out=w1[:], in_=W[P:2 * P, :])

            dt_ = cpool.tile([P, NG], _f32, tag="deg")
            norm = cpool.tile([P, NG], _f32, tag="norm")
            nc.sync.dma_start(out=dt_[:], in_=odeg[:, :])
            nc.vector.tensor_scalar_max(out=dt_[:], in0=dt_[:], scalar1=1.0)
            nc.vector.reciprocal(out=dt_[:], in_=dt_[:])
            nc.scalar.sqrt(out=norm[:], in_=dt_[:])

            for g0 in range(0, NG, HBLK):
                nb = min(HBLK, NG - g0)
                l0 = hpool.tile([P, HBLK * P], _f32, tag="l0")
                l1 = hpool.tile([P, HBLK * P], _f32, tag="l1")
                nc.sync.dma_start(out=l0[:, :nb * P],
                                  in_=hT[0, :, g0 * P:(g0 + nb) * P])
                nc.sync.dma_start(out=l1[:, :nb * P],
                                  in_=hT[1, :, g0 * P:(g0 + nb) * P])
                for j in range(nb):
                    g = g0 + j
                    acc = psum.tile([P, OUT_DIM], _f32, tag="acc")
                    nc.tensor.matmul(acc[:], l0[:, j * P:(j + 1) * P], w0[:],
                                     start=True, stop=False)
                    nc.tensor.matmul(acc[:], l1[:, j * P:(j + 1) * P], w1[:],
                                     start=False, stop=True)
                    ms = pool.tile([P, OUT_DIM], _f32, tag="ms")
                    nc.scalar.activation(
                        out=ms[:], in_=acc[:],
                        func=mybir.ActivationFunctionType.Identity,
                        scale=norm[:, g:g + 1])
                    nc.sync.dma_start(out=m[g * P:(g + 1) * P, :], in_=ms[:])
    nc.compile()
    return nc


# ---------------------------------------------------------------- launch B
def build_launch_b(meta, repeat=1):
    """meta["rounds"][i]:
      groups; q_numidx[NT]; q_choff[NT]; nch; idx_off; ch_off
      gldt: {g: (ldt_col_start, ngch)}   # ldst cols, group-major contiguous
      ggt:  {g: [gt_column, ...]}        # gather-tile column per oh chunk
    """
    nc = bacc.Bacc("TRN2", target_bir_lowering=False, debug=False,
                   num_devices=NCORES)
    tabs = [nc.dram_tensor(f"t{q}", [TROWS, OUT_DIM], _f32,
                           kind="ExternalInput") for q in range(NT)]
    gidx = nc.dram_tensor("gidx", [P, meta["tot_idx_cols"]], _i16,
                          kind="ExternalInput")
    ldst = nc.dram_tensor("ldst", [P, meta["tot_chunks"]], _bf16,
                          kind="ExternalInput")
    max_gch = meta["max_gch"]
    ideg = nc.dram_tensor("ideg", [P, NG], _f32, kind="ExternalInput")
    brep = nc.dram_tensor("brep", [P, OUT_DIM], _f32, kind="ExternalInput")
    # iotar[p, m, c] = m  (bf16) — constant compare target for one-hot builds
    iotar = nc.dram_tensor("iotar", [P, P, max_gch], _bf16,
                           kind="ExternalInput")
    out = nc.dram_tensor("out", [NG, P, OUT_DIM], _f32, kind="ExternalOutput")

    with tile.TileContext(nc) as tc:
        loop = tc.For_i(0, repeat, 1) if repeat > 1 \
            else contextlib.nullcontext()
        with loop, \
                tc.tile_pool(name="const", bufs=1) as cpool, \
                tc.tile_pool(name="gath", bufs=2) as gpool, \
                tc.tile_pool(name="conv", bufs=2) as vpool, \
                tc.tile_pool(name="meta", bufs=2) as mpool, \
                tc.tile_pool(name="onehot", bufs=3) as opool, \
                tc.tile_pool(name="epi", bufs=4) as epool, \
                tc.tile_pool(name="psum", bufs=8, space="PSUM") as psum:
            bt = cpool.tile([P, OUT_DIM], _f32, tag="b")
            it = cpool.tile([P, P, max_gch], _bf16, tag="iotar")
            nc.sync.dma_start(out=bt[:], in_=brep[:, :])
            nc.sync.dma_start(out=it[:], in_=iotar[:, :, :])

            dt_ = cpool.tile([P, NG], _f32, tag="deg")
            norm = cpool.tile([P, NG], _f32, tag="norm")
            nc.sync.dma_start(out=dt_[:], in_=ideg[:, :])
            nc.vector.tensor_scalar_max(out=dt_[:], in0=dt_[:], scalar1=1.0)
            nc.vector.reciprocal(out=dt_[:], in_=dt_[:])
            nc.scalar.sqrt(out=norm[:], in_=dt_[:])

            # persistent per-group staging: y = x - max(x); s = sum(exp)
            y_all = cpool.tile([P, NG * OUT_DIM], _f32, tag="yall")
            s_all = cpool.tile([P, NG], _f32, tag="sall")

            for ri, rnd in enumerate(meta["rounds"]):
                nch = rnd["nch"]
                nidx_cols = sum(rnd["q_numidx"]) // 16
                ixt = mpool.tile([P, nidx_cols], _i16, tag="ix")
                nc.sync.dma_start(
                    out=ixt[:],
                    in_=gidx[:, rnd["idx_off"]:rnd["idx_off"] + nidx_cols])
                ldt = mpool.tile([P, nch], _bf16, tag="ld")
                nc.sync.dma_start(
                    out=ldt[:],
                    in_=ldst[:, rnd["ch_off"]:rnd["ch_off"] + nch])

                gt = gpool.tile([P, nch, OUT_DIM], _f32, tag="gt")
                if ri < 2:
                    # first use of each rotating slot (bufs=2): clear so chunk
                    # padding never feeds NaN bit-patterns into the matmul
                    nc.gpsimd.memset(gt[:], 0.0)
                icol = 0
                for q in range(NT):
                    nq = rnd["q_numidx"][q]
                    if nq == 0:
                        continue
                    co = rnd["q_choff"][q]
                    nc.gpsimd.dma_gather(
                        out_ap=gt[:, co:co + nq // P, :],
                        in_ap=tabs[q][:, :],
                        idxs_ap=ixt[:, icol:icol + nq // 16],
                        num_idxs=nq,
                        num_idxs_reg=nq,
                        elem_size=OUT_DIM,
                        single_packet=False,
                    )
                    icol += nq // 16

                # fp32 -> bf16 rhs for full-rate PE; alternate engines by round
                gb = vpool.tile([P, nch, OUT_DIM], _bf16, tag="gb")
                if ri % 2 == 0:
                    nc.vector.tensor_copy(out=gb[:], in_=gt[:])
                else:
                    nc.scalar.activation(
                        out=gb[:], in_=gt[:],
                        func=mybir.ActivationFunctionType.Copy)

                for g in rnd["groups"]:
                    ldt0, ngch = rnd["gldt"][g]
                    gtcols = rnd["ggt"][g]
                    x = epool.tile([P, OUT_DIM], _f32, tag="x")
                    if ngch:
                        # one-hot, chunk-last: oh[k, m, c] = (ldst[k,col_c]==m)
                        # all APs keep a packed last dim -> DVE 2x mode
                        oh = opool.tile([P, P, max_gch], _bf16, tag="oh")
                        nc.vector.tensor_tensor(
                            out=oh[:, :, 0:ngch],
                            in0=_expand_mid(ldt[:, ldt0:ldt0 + ngch], P),
                            in1=it[:, :, 0:ngch],
                            op=mybir.AluOpType.is_equal)
                        acc = psum.tile([P, OUT_DIM], _f32, tag="acc")
                        for k, cg in enumerate(gtcols):
                            nc.tensor.matmul(
                                acc[:], oh[:, :, k], gb[:, cg, :],
                                start=(k == 0), stop=(k == ngch - 1))
                        nc.scalar.activation(
                            out=x[:], in_=acc[:],
                            func=mybir.ActivationFunctionType.Identity,
                            scale=norm[:, g:g + 1])
                    else:
                        nc.vector.memset(x[:], 0.0)
                    nc.vector.tensor_add(out=x[:], in0=x[:], in1=bt[:])
                    nmx = epool.tile([P, 1], _f32, tag="nmx")
                    nc.vector.tensor_reduce(out=nmx[:], in_=x[:],
                                            axis=mybir.AxisListType.X,
                                            op=mybir.AluOpType.max,
                                            negate=True)
                    nc.vector.tensor_scalar_add(
                        out=y_all[:, g * OUT_DIM:(g + 1) * OUT_DIM],
                        in0=x[:], scalar1=nmx[:, :1])
                    e = epool.tile([P, OUT_DIM], _f32, tag="e")
                    nc.scalar.activation(
                        out=e[:], in_=x[:],
                        func=mybir.ActivationFunctionType.Exp,
                        bias=nmx[:, :1], accum_out=s_all[:, g:g + 1])

            # single Ln over all groups, then finalize + store per group
            ls_all = cpool.tile([P, NG], _f32, tag="lsall")
            nc.scalar.activation(out=ls_all[:], in_=s_all[:],
                                 func=mybir.ActivationFunctionType.Ln)
            for g in range(NG):
                fin = epool.tile([P, OUT_DIM], _f32, tag="fin")
                nc.vector.tensor_scalar_sub(
                    out=fin[:],
                    in0=y_all[:, g * OUT_DIM:(g + 1) * OUT_DIM],
                    scalar1=ls_all[:, g:g + 1])
                rows = min(P, G - g * P)
                nc.sync.dma_start(out=out[g, :rows, :], in_=fin[:rows, :])
    nc.compile()
    return nc


# ------------------------------------------------------------- host prep
def _wrap_idx16(flat):
    """int16 index list -> [128, len/16] wrapped layout (16-partition groups,
    replicated across the 8 gpsimd cores)."""
    n = len(flat)
    s = n // 16
    arr = np.empty((P, s), dtype=np.int16)
    blk = flat.reshape(s, 16).T  # [16, s]
    for grp in range(8):
        arr[grp * 16:(grp + 1) * 16, :] = blk
    return arr


def prepare(h, W, b, edges):
    h = np.asarray(h, dtype=np.float32)
    W = np.asarray(W, dtype=np.float32)
    b = np.asarray(b, dtype=np.float32)
    src = np.asarray(edges[0], dtype=np.int64)
    dst = np.asarray(edges[1], dtype=np.int64)

    out_deg = np.bincount(src, minlength=N_NODES).astype(np.float32)
    in_deg = np.bincount(dst, minlength=N_NODES).astype(np.float32)

    # global m-table row for each src node (padded per-core layout)
    score = src // G
    mrow = score * GPAD + (src - score * G)
    qtab = mrow // TROWS
    lrow = (mrow - qtab * TROWS).astype(np.int16)

    dcore = dst // G
    dloc = dst - dcore * G
    grp = dloc // P
    ldst_v = (dloc - grp * P).astype(np.float32)

    # bucket = (dst-core, group, sub-table)
    bucket = (dcore * NG + grp) * NT + qtab
    order = np.argsort(bucket, kind="stable")
    bucket_s = bucket[order]
    lrow_s = lrow[order]
    ldst_s = ldst_v[order]

    nbuck = NCORES * NG * NT
    counts = np.bincount(bucket_s, minlength=nbuck).reshape(NCORES, NG, NT)
    starts = np.zeros(nbuck + 1, dtype=np.int64)
    np.cumsum(counts.reshape(-1), out=starts[1:])

    # uniform capacity per (group, sub-table): max over cores, ceil to 128
    cap = counts.max(axis=0)                      # [NG, NT]
    cap128 = ((cap + P - 1) // P) * P             # [NG, NT]

    # round structure (uniform across cores)
    # gather tile columns: (q, g, chunk) order; ldst columns: (g, q, chunk)
    rounds = []
    idx_off = 0
    ch_off = 0
    for r0 in range(0, NG, ROUND_G):
        gs = list(range(r0, min(r0 + ROUND_G, NG)))
        q_numidx, q_choff = [], []
        gt_col = {}          # (g, q) -> gather-tile column base (within round)
        cursor = 0
        for q in range(NT):
            q_choff.append(cursor)
            tot = 0
            for g in gs:
                c = int(cap128[g, q])
                gt_col[(g, q)] = cursor
                cursor += c // P
                tot += c
            q_numidx.append(tot)
        gldt, ggt = {}, {}
        lcur = 0
        for g in gs:
            cols = []
            for q in range(NT):
                cols.extend(range(gt_col[(g, q)],
                                  gt_col[(g, q)] + int(cap128[g, q]) // P))
            gldt[g] = (lcur, len(cols))
            ggt[g] = cols
            lcur += len(cols)
        rounds.append(dict(groups=gs, q_numidx=q_numidx, q_choff=q_choff,
                           nch=cursor, idx_off=idx_off, ch_off=ch_off,
                           gldt=gldt, ggt=ggt, gt_col=gt_col))
        idx_off += sum(q_numidx) // 16
        ch_off += cursor
    max_gch = max(rnd["gldt"][g][1] for rnd in rounds for g in rnd["groups"])
    meta = dict(rounds=rounds, tot_idx_cols=idx_off, tot_chunks=ch_off,
                max_gch=max_gch)

    # per-core gidx / ldst arrays
    gidx_cores = []
    ldst_cores = []
    for c in range(NCORES):
        flat_idx = np.zeros(idx_off * 16, dtype=np.int16)
        ld = np.full((P, ch_off), PAD_LDST, dtype=np.float32)
        for rnd in rounds:
            pos = rnd["idx_off"] * 16
            for q in range(NT):
                for g in rnd["groups"]:
                    bid = (c * NG + g) * NT + q
                    s0, s1 = starts[bid], starts[bid + 1]
                    n = s1 - s0
                    capq = int(cap128[g, q])
                    flat_idx[pos:pos + n] = lrow_s[s0:s1]
                    pos += capq
                    # ldst column base: group-major layout
                    qch0 = sum(int(cap128[g, q2]) // P for q2 in range(q))
                    base = rnd["ch_off"] + rnd["gldt"][g][0] + qch0
                    j = np.arange(n)
                    ld[j % P, base + j // P] = ldst_s[s0:s1]
        gidx_cores.append(_wrap_idx16(flat_idx))
        ldst_cores.append(ld.astype(ml_dtypes.bfloat16))

    # degree tiles [128, NG] (partition = node % 128 within group)
    def deg_tile(deg):
        tiles = []
        for c in range(NCORES):
            d = np.ones(GPAD, dtype=np.float32)
            d[:G] = deg[c * G:(c + 1) * G]
            tiles.append(d.reshape(NG, P).T.copy())
        return tiles

    odeg_tiles = deg_tile(out_deg)
    ideg_tiles = deg_tile(in_deg)

    hT_cores = []
    for c in range(NCORES):
        hp = np.zeros((GPAD, IN_DIM), dtype=np.float32)
        hp[:G] = h[c * G:(c + 1) * G]
        # [2, 128, GPAD]: k-halves, contiguous along nodes for wide DMAs
        ht = np.ascontiguousarray(hp.T.reshape(2, P, GPAD))
        hT_cores.append(ht)

    brep = np.broadcast_to(b, (P, OUT_DIM)).copy()
    # iotar[p, m, c] = m
    iotar = np.broadcast_to(
        np.arange(P, dtype=np.float32)[None, :, None],
        (P, P, max_gch)).astype(ml_dtypes.bfloat16).copy()

    return dict(meta=meta, gidx=gidx_cores, ldst=ldst_cores,
                odeg=odeg_tiles, ideg=ideg_tiles, hT=hT_cores,
                W=W, brep=brep, iotar=iotar)


_cache = {}


def _get_programs(meta):
    if "a" not in _cache:
        _cache["a"] = build_launch_a()
    if "b" not in _cache:
        _cache["b"] = build_launch_b(meta)
    return _cache["a"], _cache["b"]


def run_launch_a(nc_a, prep):
    in_maps = [{"hT": prep["hT"][c], "W": prep["W"], "odeg": prep["odeg"][c]}
               for c in range(NCORES)]
    res = run_bass_kernel_spmd(nc_a, in_maps, list(range(NCORES)))
    return [r["m"] for r in res.results]


def run_launch_b(nc_b, prep, m_shards):
    m_full = np.concatenate(m_shards, axis=0)  # [NCORES*GPAD, 64]
    tabs = {f"t{q}": np.ascontiguousarray(m_full[q * TROWS:(q + 1) * TROWS])
            for q in range(NT)}
    in_maps = [dict(tabs, gidx=prep["gidx"][c], ldst=prep["ldst"][c],
                    ideg=prep["ideg"][c], brep=prep["brep"],
                    iotar=prep["iotar"]) for c in range(NCORES)]
    res = run_bass_kernel_spmd(nc_b, in_maps, list(range(NCORES)))
    return np.concatenate(
        [r["out"].reshape(GPAD, OUT_DIM)[:G] for r in res.results], axis=0)


def kernel(h, W, b, edges):
    prep = prepare(h, W, b, edges)
    nc_a, nc_b = _get_programs(prep["meta"])
    m_shards = run_launch_a(nc_a, prep)
    out = run_launch_b(nc_b, prep, m_shards)
    return out.astype(np.float32)



# revision 3
# speedup vs baseline: 2.1250x; 2.1250x over previous
"""GraphConv (DGL norm='both') + log_softmax on 8 Trainium2 NeuronCores.

Strategy (per sharding hint): partition nodes across the 8 cores by range.
  Launch A (per core): project its 12500-node slice m = (h @ W) * out_deg^-1/2
  in bf16 (PE bf16, PSUM f32 accumulate).
  Host: concatenate the 8 projected shards into a replicated gather table,
  viewed as PAIRED rows [50176, 128] bf16 so the table row stride is 256 B
  (DMA descriptor encoding granularity) while each gather moves only the
  needed 128-B half-row (the pair parity selects a 64-col offset).
  Launch B (per core): for its 12500 dst nodes, gather m[src] half-rows for
  all in-edges (dma_gather, edges pre-sorted by dst group), segment-sum via
  one-hot matmuls accumulating in PSUM, then norm/bias/log_softmax.

Degrees and the sorted/padded edge metadata are sharding-prep computed on the
host (numpy); all FLOPs on h/W/b/m (projection, normalization, aggregation,
softmax) run on device.
"""

import contextlib

import numpy as np
import ml_dtypes

import concourse.bass as bass
import concourse.bacc as bacc
import concourse.mybir as mybir
import concourse.tile as tile
from concourse.bass import AP
from concourse.bass_utils import run_bass_kernel_spmd

P = 128
N_NODES = 100000
N_EDGES = 3200000
IN_DIM = 256
OUT_DIM = 64
NCORES = 8
G = N_NODES // NCORES            # 12500 nodes per core
NG = (G + P - 1) // P            # 98 groups of 128 dst nodes (last has 84)
GPAD = NG * P                    # 12544
NPAIR = (NCORES * GPAD) // 2     # 50176 paired table rows
NT = 2                           # sub-tables (int16 index limit)
TROWS = NPAIR // NT              # 25088 rows per sub-table
NCLS = NT * 2                    # gather classes: (sub-table, parity)
ROUND_G = 8                      # dst groups per gather round
HBLK = 16                        # dst groups per hT load in launch A
PAD_LDST = 200.0                 # local-dst for padded edges (>127, exact bf16)

_f32 = mybir.dt.float32
_bf16 = mybir.dt.bfloat16
_i16 = mybir.dt.int16


def _expand_mid(ap, n):
    """[P, C] AP -> [P, n, C] AP repeating each partition row n times
    (middle broadcast keeps the last dim packed, so DVE 2x mode applies)."""
    (ps, pc), (cs, cc) = ap.ap[0], ap.ap[1]
    return AP(ap.tensor, ap.offset, [[ps, pc], [0, n], [cs, cc]])


def _expand_last(ap, n):
    """[P, C] AP -> [P, C, n] AP repeating each element n times along a new
    innermost (stride-0) dim."""
    (ps, pc), (cs, cc) = ap.ap[0], ap.ap[1]
    return AP(ap.tensor, ap.offset, [[ps, pc], [cs, cc], [0, n]])


def _dma_gather_half(eng, out_ap, in_ap, idxs_ap, num_idxs, elem_size,
                     elem_step, single_packet=False):
    """dma_gather with a sub-256B payload (row stride must stay 256B-aligned:
    elem_step * dtype_size % 256 == 0). Same IR as bass's dma_gather helper,
    minus its payload-granularity assert (the HW descriptor only constrains
    the stride; the payload is free-form)."""
    stride_bytes = elem_step * mybir.dt.size(in_ap.dtype)
    assert stride_bytes % 256 == 0
    assert in_ap.ap[0][0] == elem_step
    return eng.add_instruction(
        mybir.InstDMAGatherAnt(
            name=eng.bass.get_next_instruction_name(),
            ins=[*eng.lower_ap_dma(in_ap, for_custom_bir_dma=True),
                 eng.lower_ap(idxs_ap),
                 eng.lower_val_access(eng.to_reg(num_idxs))],
            outs=[eng.lower_ap(out_ap)],
            transpose=False,
            num_idxs=num_idxs,
            elem_size=elem_size,
            stride_bytes_256=stride_bytes // 256,
            gen_mode=0,
            single_packet=single_packet,
            queue_num=0,
            sbuf_tokens_per_rank=0,
            sbuf_free_dim_per_rank=0,
            sbuf_free_dim_pad_per_rank=0,
            sbuf_byte_offset=0,
        ))


# ---------------------------------------------------------------- launch A
def build_launch_a(repeat=1):
    nc = bacc.Bacc("TRN2", target_bir_lowering=False, debug=False,
                   num_devices=NCORES)
    hT = nc.dram_tensor("hT", [2, P, GPAD], _bf16, kind="ExternalInput")
    W = nc.dram_tensor("W", [2, P, OUT_DIM], _bf16, kind="ExternalInput")
    odeg = nc.dram_tensor("odeg", [P, NG], _f32, kind="ExternalInput")
    # partition-major projected features: m[p, g*64+f] = m_row(g*128+p, f)
    m = nc.dram_tensor("m", [P, NG * OUT_DIM], _bf16, kind="ExternalOutput")

    with tile.TileContext(nc) as tc:
        loop = tc.For_i(0, repeat, 1) if repeat > 1 \
            else contextlib.nullcontext()
        with loop, \
                tc.tile_pool(name="const", bufs=1) as cpool, \
                tc.tile_pool(name="hblk", bufs=2) as hpool, \
                tc.tile_pool(name="mstage", bufs=2) as mpool, \
                tc.tile_pool(name="psum", bufs=8, space="PSUM") as psum:
            w0 = cpool.tile([P, OUT_DIM], _bf16, tag="w0")
            w1 = cpool.tile([P, OUT_DIM], _bf16, tag="w1")
            nc.sync.dma_start(out=w0[:], in_=W[0, :, :])
            nc.sync.dma_start(out=w1[:], in_=W[1, :, :])

            dt_ = cpool.tile([P, NG], _f32, tag="deg")
            norm = cpool.tile([P, NG], _f32, tag="norm")
            nc.sync.dma_start(out=dt_[:], in_=odeg[:, :])
            nc.vector.tensor_scalar_max(out=dt_[:], in0=dt_[:], scalar1=1.0)
            nc.vector.reciprocal(out=dt_[:], in_=dt_[:])
            nc.scalar.sqrt(out=norm[:], in_=dt_[:])

            for g0 in range(0, NG, HBLK):
                nb = min(HBLK, NG - g0)
                l0 = hpool.tile([P, HBLK * P], _bf16, tag="l0")
                l1 = hpool.tile([P, HBLK * P], _bf16, tag="l1")
                nc.sync.dma_start(out=l0[:, :nb * P],
                                  in_=hT[0, :, g0 * P:(g0 + nb) * P])
                nc.sync.dma_start(out=l1[:, :nb * P],
                                  in_=hT[1, :, g0 * P:(g0 + nb) * P])
                ms = mpool.tile([P, HBLK * OUT_DIM], _bf16, tag="ms")
                for j in range(nb):
                    g = g0 + j
                    acc = psum.tile([P, OUT_DIM], _f32, tag="acc")
                    nc.tensor.matmul(acc[:], l0[:, j * P:(j + 1) * P], w0[:],
                                     start=True, stop=False)
                    nc.tensor.matmul(acc[:], l1[:, j * P:(j + 1) * P], w1[:],
                                     start=False, stop=True)
                    nc.scalar.activation(
                        out=ms[:, j * OUT_DIM:(j + 1) * OUT_DIM], in_=acc[:],
                        func=mybir.ActivationFunctionType.Identity,
                        scale=norm[:, g:g + 1])
                nc.sync.dma_start(
                    out=m[:, g0 * OUT_DIM:(g0 + nb) * OUT_DIM],
                    in_=ms[:, :nb * OUT_DIM])
    nc.compile()
    return nc


# ---------------------------------------------------------------- launch B
def build_launch_b(meta, repeat=1):
    """meta["rounds"][i]:
      groups; q_numidx[NCLS]; q_choff[NCLS]; nch; idx_off; ch_off
      gldt: {g: (ldt_col_start, ngch)}   # ldst cols, group-major contiguous
      ggt:  {g: [gt_column, ...]}        # gather-tile column per oh chunk
    """
    nc = bacc.Bacc("TRN2", target_bir_lowering=False, debug=False,
                   num_devices=NCORES)
    tabs = [nc.dram_tensor(f"t{q}", [TROWS, 2 * OUT_DIM], _bf16,
                           kind="ExternalInput") for q in range(NT)]
    gidx = nc.dram_tensor("gidx", [P, meta["tot_idx_cols"]], _i16,
                          kind="ExternalInput")
    ldst = nc.dram_tensor("ldst", [P, meta["tot_chunks"]], _bf16,
                          kind="ExternalInput")
    max_gch = meta["max_gch"]
    ideg = nc.dram_tensor("ideg", [P, NG], _f32, kind="ExternalInput")
    brep = nc.dram_tensor("brep", [P, OUT_DIM], _f32, kind="ExternalInput")
    # iotar[p, m, c] = m  (bf16) — constant compare target for one-hot builds
    iotar = nc.dram_tensor("iotar", [P, P, max_gch], _bf16,
                           kind="ExternalInput")
    # partition-major output: out[p, g, f] = result(g*128+p, f)
    out = nc.dram_tensor("out", [P, NG, OUT_DIM], _f32, kind="ExternalOutput")

    with tile.TileContext(nc) as tc:
        loop = tc.For_i(0, repeat, 1) if repeat > 1 \
            else contextlib.nullcontext()
        with loop, \
                tc.tile_pool(name="const", bufs=1) as cpool, \
                tc.tile_pool(name="gath", bufs=2) as gpool, \
                tc.tile_pool(name="meta", bufs=2) as mpool, \
                tc.tile_pool(name="onehot", bufs=3) as opool, \
                tc.tile_pool(name="epi", bufs=3) as epool, \
                tc.tile_pool(name="psum", bufs=8, space="PSUM") as psum:
            bt = cpool.tile([P, OUT_DIM], _f32, tag="b")
            it = cpool.tile([P, P, max_gch], _bf16, tag="iotar")
            nc.sync.dma_start(out=bt[:], in_=brep[:, :])
            nc.sync.dma_start(out=it[:], in_=iotar[:, :, :])

            dt_ = cpool.tile([P, NG], _f32, tag="deg")
            norm = cpool.tile([P, NG], _f32, tag="norm")
            nc.sync.dma_start(out=dt_[:], in_=ideg[:, :])
            nc.vector.tensor_scalar_max(out=dt_[:], in0=dt_[:], scalar1=1.0)
            nc.vector.reciprocal(out=dt_[:], in_=dt_[:])
            nc.scalar.sqrt(out=norm[:], in_=dt_[:])

            # persistent per-group staging: y = x - max(x); s = sum(exp)
            y_all = cpool.tile([P, NG, OUT_DIM], _f32, tag="yall")
            s_all = cpool.tile([P, NG], _f32, tag="sall")

            for ri, rnd in enumerate(meta["rounds"]):
                gs = rnd["groups"]
                rg = len(gs)
                nch = rnd["nch"]
                nidx_cols = sum(rnd["q_numidx"]) // 16
                ixt = mpool.tile([P, nidx_cols], _i16, tag="ix")
                nc.sync.dma_start(
                    out=ixt[:],
                    in_=gidx[:, rnd["idx_off"]:rnd["idx_off"] + nidx_cols])
                ldt = mpool.tile([P, nch], _bf16, tag="ld")
                nc.sync.dma_start(
                    out=ldt[:],
                    in_=ldst[:, rnd["ch_off"]:rnd["ch_off"] + nch])

                gt = gpool.tile([P, nch, OUT_DIM], _bf16, tag="gt")
                icol = 0
                for q in range(NT):
                    for par in range(2):
                        nq = rnd["q_numidx"][q * 2 + par]
                        if nq == 0:
                            continue
                        co = rnd["q_choff"][q * 2 + par]
                        _dma_gather_half(
                            nc.gpsimd,
                            out_ap=gt[:, co:co + nq // P, :],
                            in_ap=tabs[q][:, par * OUT_DIM:(par + 1) * OUT_DIM],
                            idxs_ap=ixt[:, icol:icol + nq // 16],
                            num_idxs=nq,
                            elem_size=OUT_DIM,
                            elem_step=2 * OUT_DIM,
                            single_packet=False,
                        )
                        icol += nq // 16

                xr = epool.tile([P, ROUND_G, OUT_DIM], _f32, tag="xr")
                for i, g in enumerate(gs):
                    ldt0, ngch = rnd["gldt"][g]
                    gtcols = rnd["ggt"][g]
                    # one-hot, chunk-last: oh[k, m, c] = (ldst[k,col_c]==m)
                    # all APs keep a packed last dim -> DVE 2x mode
                    oh = opool.tile([P, P, max_gch], _bf16, tag="oh")
                    nc.vector.tensor_tensor(
                        out=oh[:, :, 0:ngch],
                        in0=_expand_mid(ldt[:, ldt0:ldt0 + ngch], P),
                        in1=it[:, :, 0:ngch],
                        op=mybir.AluOpType.is_equal)
                    acc = psum.tile([P, OUT_DIM], _f32, tag="acc")
                    for k, cg in enumerate(gtcols):
                        nc.tensor.matmul(
                            acc[:], oh[:, :, k], gt[:, cg, :],
                            start=(k == 0), stop=(k == ngch - 1))
                    nc.scalar.activation(
                        out=xr[:, i, :], in_=acc[:],
                        func=mybir.ActivationFunctionType.Identity,
                        scale=norm[:, g:g + 1])

                g0 = gs[0]
                # batched epilogue for the round's rg groups
                nc.vector.tensor_tensor(
                    out=xr[:, :rg, :], in0=xr[:, :rg, :],
                    in1=_expand_mid(bt[:, :], rg),
                    op=mybir.AluOpType.add)
                nmx = epool.tile([P, ROUND_G], _f32, tag="nmx")
                nc.vector.tensor_reduce(out=nmx[:, :rg], in_=xr[:, :rg, :],
                                        axis=mybir.AxisListType.X,
                                        op=mybir.AluOpType.max,
                                        negate=True)
                nc.vector.tensor_tensor(
                    out=y_all[:, g0:g0 + rg, :], in0=xr[:, :rg, :],
                    in1=_expand_last(nmx[:, :rg], OUT_DIM),
                    op=mybir.AluOpType.add)
                e = epool.tile([P, ROUND_G, OUT_DIM], _f32, tag="e")
                for i, g in enumerate(gs):
                    nc.scalar.activation(
                        out=e[:, i, :], in_=y_all[:, g, :],
                        func=mybir.ActivationFunctionType.Exp,
                        accum_out=s_all[:, g:g + 1])

            # single Ln over all groups, then finalize + store per round
            ls_all = cpool.tile([P, NG], _f32, tag="lsall")
            nc.scalar.activation(out=ls_all[:], in_=s_all[:],
                                 func=mybir.ActivationFunctionType.Ln)
            for rnd in meta["rounds"]:
                gs = rnd["groups"]
                g0 = gs[0]
                rg = len(gs)
                fin = epool.tile([P, ROUND_G, OUT_DIM], _f32, tag="fin")
                nc.vector.tensor_tensor(
                    out=fin[:, :rg, :], in0=y_all[:, g0:g0 + rg, :],
                    in1=_expand_last(ls_all[:, g0:g0 + rg], OUT_DIM),
                    op=mybir.AluOpType.subtract)
                nc.sync.dma_start(out=out[:, g0:g0 + rg, :],
                                  in_=fin[:, :rg, :])
    nc.compile()
    return nc


# ------------------------------------------------------------- host prep
def _wrap_idx16(flat):
    """int16 index list -> [128, len/16] wrapped layout (16-partition groups,
    replicated across the 8 gpsimd cores)."""
    n = len(flat)
    s = n // 16
    arr = np.empty((P, s), dtype=np.int16)
    blk = flat.reshape(s, 16).T  # [16, s]
    for grp in range(8):
        arr[grp * 16:(grp + 1) * 16, :] = blk
    return arr


def prepare(h, W, b, edges):
    h = np.asarray(h, dtype=np.float32)
    W = np.asarray(W, dtype=np.float32)
    b = np.asarray(b, dtype=np.float32)
    src = np.asarray(edges[0], dtype=np.int64)
    dst = np.asarray(edges[1], dtype=np.int64)

    out_deg = np.bincount(src, minlength=N_NODES).astype(np.float32)
    in_deg = np.bincount(dst, minlength=N_NODES).astype(np.float32)

    # global m-table row for each src node (padded per-core layout), then
    # paired-row coordinates: pair index + parity -> (sub-table, class)
    score = src // G
    mrow = score * GPAD + (src - score * G)
    pair = mrow >> 1
    par = mrow & 1
    qtab = pair // TROWS
    lrow = (pair - qtab * TROWS).astype(np.int16)
    cls = qtab * 2 + par

    dcore = dst // G
    dloc = dst - dcore * G
    grp = dloc // P
    ldst_v = (dloc - grp * P).astype(np.float32)

    # bucket = (dst-core, group, class)
    bucket = (dcore * NG + grp) * NCLS + cls
    order = np.argsort(bucket, kind="stable")
    bucket_s = bucket[order]
    lrow_s = lrow[order]
    ldst_s = ldst_v[order]

    nbuck = NCORES * NG * NCLS
    counts = np.bincount(bucket_s, minlength=nbuck).reshape(NCORES, NG, NCLS)
    starts = np.zeros(nbuck + 1, dtype=np.int64)
    np.cumsum(counts.reshape(-1), out=starts[1:])

    # uniform capacity per (group, class): max over cores, ceil to 128
    cap = counts.max(axis=0)                      # [NG, NCLS]
    cap128 = ((cap + P - 1) // P) * P             # [NG, NCLS]

    # round structure (uniform across cores)
    # gather tile columns: (class, g, chunk) order; ldst: (g, class, chunk)
    rounds = []
    idx_off = 0
    ch_off = 0
    for r0 in range(0, NG, ROUND_G):
        gs = list(range(r0, min(r0 + ROUND_G, NG)))
        q_numidx, q_choff = [], []
        gt_col = {}          # (g, c) -> gather-tile column base (within round)
        cursor = 0
        for c in range(NCLS):
            q_choff.append(cursor)
            tot = 0
            for g in gs:
                cp = int(cap128[g, c])
                gt_col[(g, c)] = cursor
                cursor += cp // P
                tot += cp
            q_numidx.append(tot)
        gldt, ggt = {}, {}
        lcur = 0
        for g in gs:
            cols = []
            for c in range(NCLS):
                cols.extend(range(gt_col[(g, c)],
                                  gt_col[(g, c)] + int(cap128[g, c]) // P))
            gldt[g] = (lcur, len(cols))
            ggt[g] = cols
            lcur += len(cols)
        rounds.append(dict(groups=gs, q_numidx=q_numidx, q_choff=q_choff,
                           nch=cursor, idx_off=idx_off, ch_off=ch_off,
                           gldt=gldt, ggt=ggt, gt_col=gt_col))
        idx_off += sum(q_numidx) // 16
        ch_off += cursor
    max_gch = max(rnd["gldt"][g][1] for rnd in rounds for g in rnd["groups"])
    meta = dict(rounds=rounds, tot_idx_cols=idx_off, tot_chunks=ch_off,
                max_gch=max_gch)

    # per-core gidx / ldst arrays
    gidx_cores = []
    ldst_cores = []
    for c0 in range(NCORES):
        flat_idx = np.zeros(idx_off * 16, dtype=np.int16)
        ld = np.full((P, ch_off), PAD_LDST, dtype=np.float32)
        for rnd in rounds:
            pos = rnd["idx_off"] * 16
            for c in range(NCLS):
                for g in rnd["groups"]:
                    bid = (c0 * NG + g) * NCLS + c
                    s0, s1 = starts[bid], starts[bid + 1]
                    n = s1 - s0
                    capq = int(cap128[g, c])
                    flat_idx[pos:pos + n] = lrow_s[s0:s1]
                    pos += capq
                    # ldst column base: group-major layout
                    qch0 = sum(int(cap128[g, c2]) // P for c2 in range(c))
                    base = rnd["ch_off"] + rnd["gldt"][g][0] + qch0
                    j = np.arange(n)
                    ld[j % P, base + j // P] = ldst_s[s0:s1]
        gidx_cores.append(_wrap_idx16(flat_idx))
        ldst_cores.append(ld.astype(ml_dtypes.bfloat16))

    # degree tiles [128, NG] (partition = node % 128 within group)
    def deg_tile(deg):
        tiles = []
        for c in range(NCORES):
            d = np.ones(GPAD, dtype=np.float32)
            d[:G] = deg[c * G:(c + 1) * G]
            tiles.append(d.reshape(NG, P).T.copy())
        return tiles

    odeg_tiles = deg_tile(out_deg)
    ideg_tiles = deg_tile(in_deg)

    hT_cores = []
    for c in range(NCORES):
        hp = np.zeros((GPAD, IN_DIM), dtype=np.float32)
        hp[:G] = h[c * G:(c + 1) * G]
        # [2, 128, GPAD]: k-halves, contiguous along nodes for wide DMAs
        ht = np.ascontiguousarray(hp.T.reshape(2, P, GPAD))
        hT_cores.append(ht.astype(ml_dtypes.bfloat16))

    wt = np.ascontiguousarray(W.reshape(2, P, OUT_DIM)).astype(
        ml_dtypes.bfloat16)
    brep = np.broadcast_to(b, (P, OUT_DIM)).copy()
    # iotar[p, m, c] = m
    iotar = np.broadcast_to(
        np.arange(P, dtype=np.float32)[None, :, None],
        (P, P, max_gch)).astype(ml_dtypes.bfloat16).copy()

    return dict(meta=meta, gidx=gidx_cores, ldst=ldst_cores,
                odeg=odeg_tiles, ideg=ideg_tiles, hT=hT_cores,
                W=wt, brep=brep, iotar=iotar)


_cache = {}


def _get_programs(meta):
    if "a" not in _cache:
        _cache["a"] = build_launch_a()
    if "b" not in _cache:
        _cache["b"] = build_launch_b(meta)
    return _cache["a"], _cache["b"]


def run_launch_a(nc_a, prep):
    in_maps = [{"hT": prep["hT"][c], "W": prep["W"], "odeg": prep["odeg"][c]}
               for c in range(NCORES)]
    res = run_bass_kernel_spmd(nc_a, in_maps, list(range(NCORES)))
    # m[p, g*64+f] -> rows (g*128+p, f)
    shards = []
    for r in res.results:
        md = np.asarray(r["m"]).reshape(P, NG, OUT_DIM)
        shards.append(md.transpose(1, 0, 2).reshape(GPAD, OUT_DIM))
    return shards


def run_launch_b(nc_b, prep, m_shards):
    m_full = np.concatenate(m_shards, axis=0)  # [NCORES*GPAD, 64] bf16
    mp = m_full.reshape(NPAIR, 2 * OUT_DIM)    # paired rows, 256 B stride
    tabs = {f"t{q}": np.ascontiguousarray(mp[q * TROWS:(q + 1) * TROWS])
            for q in range(NT)}
    in_maps = [dict(tabs, gidx=prep["gidx"][c], ldst=prep["ldst"][c],
                    ideg=prep["ideg"][c], brep=prep["brep"],
                    iotar=prep["iotar"]) for c in range(NCORES)]
    res = run_bass_kernel_spmd(nc_b, in_maps, list(range(NCORES)))
    outs = []
    for r in res.results:
        od = np.asarray(r["out"])  # [P, NG, 64]
        outs.append(od.transpose(1, 0, 2).reshape(GPAD, OUT_DIM)[:G])
    return np.concatenate(outs, axis=0)


def kernel(h, W, b, edges):
    prep = prepare(h, W, b, edges)
    nc_a, nc_b = _get_programs(prep["meta"])
    m_shards = run_launch_a(nc_a, prep)
    out = run_launch_b(nc_b, prep, m_shards)
    return out.astype(np.float32)


# revision 8
# speedup vs baseline: 2.1288x; 1.0018x over previous
"""GraphConv (DGL norm='both') + log_softmax on 8 Trainium2 NeuronCores.

Strategy (per sharding hint): partition nodes across the 8 cores by range.
  Launch A (per core): project its 12500-node slice m = (h @ W) * out_deg^-1/2
  in bf16 (PE bf16, PSUM f32 accumulate).
  Host: concatenate the 8 projected shards into a replicated gather table,
  viewed as PAIRED rows [50176, 128] bf16 so the table row stride is 256 B
  (DMA descriptor encoding granularity) while each gather moves only the
  needed 128-B half-row (the pair parity selects a 64-col offset).
  Launch B (per core): for its 12500 dst nodes, gather m[src] half-rows for
  all in-edges (dma_gather, edges pre-sorted by dst group), segment-sum via
  one-hot matmuls accumulating in PSUM, then norm/bias/log_softmax.

Degrees and the sorted/padded edge metadata are sharding-prep computed on the
host (numpy); all FLOPs on h/W/b/m (projection, normalization, aggregation,
softmax) run on device.
"""

import contextlib

import numpy as np
import ml_dtypes

import concourse.bass as bass
import concourse.bacc as bacc
import concourse.mybir as mybir
import concourse.tile as tile
from concourse.bass import AP
from concourse.bass_utils import run_bass_kernel_spmd

P = 128
N_NODES = 100000
N_EDGES = 3200000
IN_DIM = 256
OUT_DIM = 64
NCORES = 8
G = N_NODES // NCORES            # 12500 nodes per core
NG = (G + P - 1) // P            # 98 groups of 128 dst nodes (last has 84)
GPAD = NG * P                    # 12544
NPAIR = (NCORES * GPAD) // 2     # 50176 paired table rows
NT = 2                           # sub-tables (int16 index limit)
TROWS = NPAIR // NT              # 25088 rows per sub-table
NCLS = NT * 2                    # gather classes: (sub-table, parity)
ROUND_G = 8                      # dst groups per gather round
HBLK = 16                        # dst groups per hT load in launch A
PAD_LDST = 200.0                 # local-dst for padded edges (>127, exact bf16)

_f32 = mybir.dt.float32
_bf16 = mybir.dt.bfloat16
_i16 = mybir.dt.int16


def _expand_mid(ap, n):
    """[P, C] AP -> [P, n, C] AP repeating each partition row n times
    (middle broadcast keeps the last dim packed, so DVE 2x mode applies)."""
    (ps, pc), (cs, cc) = ap.ap[0], ap.ap[1]
    return AP(ap.tensor, ap.offset, [[ps, pc], [0, n], [cs, cc]])


def _expand_last(ap, n):
    """[P, C] AP -> [P, C, n] AP repeating each element n times along a new
    innermost (stride-0) dim."""
    (ps, pc), (cs, cc) = ap.ap[0], ap.ap[1]
    return AP(ap.tensor, ap.offset, [[ps, pc], [cs, cc], [0, n]])


def _dma_gather_half(eng, out_ap, in_ap, idxs_ap, num_idxs, elem_size,
                     elem_step, single_packet=False):
    """dma_gather with a sub-256B payload (row stride must stay 256B-aligned:
    elem_step * dtype_size % 256 == 0). Same IR as bass's dma_gather helper,
    minus its payload-granularity assert (the HW descriptor only constrains
    the stride; the payload is free-form)."""
    stride_bytes = elem_step * mybir.dt.size(in_ap.dtype)
    assert stride_bytes % 256 == 0
    assert in_ap.ap[0][0] == elem_step
    return eng.add_instruction(
        mybir.InstDMAGatherAnt(
            name=eng.bass.get_next_instruction_name(),
            ins=[*eng.lower_ap_dma(in_ap, for_custom_bir_dma=True),
                 eng.lower_ap(idxs_ap),
                 eng.lower_val_access(eng.to_reg(num_idxs))],
            outs=[eng.lower_ap(out_ap)],
            transpose=False,
            num_idxs=num_idxs,
            elem_size=elem_size,
            stride_bytes_256=stride_bytes // 256,
            gen_mode=0,
            single_packet=single_packet,
            queue_num=0,
            sbuf_tokens_per_rank=0,
            sbuf_free_dim_per_rank=0,
            sbuf_free_dim_pad_per_rank=0,
            sbuf_byte_offset=0,
        ))


# ---------------------------------------------------------------- launch A
def build_launch_a(repeat=1):
    nc = bacc.Bacc("TRN2", target_bir_lowering=False, debug=False,
                   num_devices=NCORES)
    hT = nc.dram_tensor("hT", [2, P, GPAD], _bf16, kind="ExternalInput")
    W = nc.dram_tensor("W", [2, P, OUT_DIM], _bf16, kind="ExternalInput")
    odeg = nc.dram_tensor("odeg", [P, NG], _f32, kind="ExternalInput")
    # partition-major projected features: m[p, g*64+f] = m_row(g*128+p, f)
    m = nc.dram_tensor("m", [P, NG * OUT_DIM], _bf16, kind="ExternalOutput")

    with tile.TileContext(nc) as tc:
        loop = tc.For_i(0, repeat, 1) if repeat > 1 \
            else contextlib.nullcontext()
        with loop, \
                tc.tile_pool(name="const", bufs=1) as cpool, \
                tc.tile_pool(name="hblk", bufs=2) as hpool, \
                tc.tile_pool(name="mstage", bufs=2) as mpool, \
                tc.tile_pool(name="psum", bufs=8, space="PSUM") as psum:
            w0 = cpool.tile([P, OUT_DIM], _bf16, tag="w0")
            w1 = cpool.tile([P, OUT_DIM], _bf16, tag="w1")
            nc.sync.dma_start(out=w0[:], in_=W[0, :, :])
            nc.sync.dma_start(out=w1[:], in_=W[1, :, :])

            dt_ = cpool.tile([P, NG], _f32, tag="deg")
            norm = cpool.tile([P, NG], _f32, tag="norm")
            nc.sync.dma_start(out=dt_[:], in_=odeg[:, :])
            nc.vector.tensor_scalar_max(out=dt_[:], in0=dt_[:], scalar1=1.0)
            nc.vector.reciprocal(out=dt_[:], in_=dt_[:])
            nc.scalar.sqrt(out=norm[:], in_=dt_[:])

            for g0 in range(0, NG, HBLK):
                nb = min(HBLK, NG - g0)
                l0 = hpool.tile([P, HBLK * P], _bf16, tag="l0")
                l1 = hpool.tile([P, HBLK * P], _bf16, tag="l1")
                nc.sync.dma_start(out=l0[:, :nb * P],
                                  in_=hT[0, :, g0 * P:(g0 + nb) * P])
                nc.scalar.dma_start(out=l1[:, :nb * P],
                                    in_=hT[1, :, g0 * P:(g0 + nb) * P])
                ms = mpool.tile([P, HBLK, OUT_DIM], _bf16, tag="ms")
                # 8 groups per PSUM bank; norm scaling batched on DVE
                for j0 in range(0, nb, 8):
                    nj = min(8, nb - j0)
                    acc8 = psum.tile([P, 8, OUT_DIM], _f32, tag="acc8")
                    for j in range(j0, j0 + nj):
                        nc.tensor.matmul(acc8[:, j - j0, :],
                                         l0[:, j * P:(j + 1) * P], w0[:],
                                         start=True, stop=False)
                        nc.tensor.matmul(acc8[:, j - j0, :],
                                         l1[:, j * P:(j + 1) * P], w1[:],
                                         start=False, stop=True)
                    nc.vector.tensor_tensor(
                        out=ms[:, j0:j0 + nj, :], in0=acc8[:, :nj, :],
                        in1=_expand_last(norm[:, g0 + j0:g0 + j0 + nj],
                                         OUT_DIM),
                        op=mybir.AluOpType.mult)
                nc.gpsimd.dma_start(
                    out=m[:, g0 * OUT_DIM:(g0 + nb) * OUT_DIM],
                    in_=ms[:, :nb, :])
    nc.compile()
    return nc


# ---------------------------------------------------------------- launch B
def build_launch_b(meta, repeat=1):
    """meta["rounds"][i]:
      groups; q_numidx[NCLS]; q_choff[NCLS]; nch; idx_off; ch_off
      gldt: {g: (ldt_col_start, ngch)}   # ldst cols, group-major contiguous
      ggt:  {g: [gt_column, ...]}        # gather-tile column per oh chunk
    """
    nc = bacc.Bacc("TRN2", target_bir_lowering=False, debug=False,
                   num_devices=NCORES)
    tabs = [nc.dram_tensor(f"t{q}", [TROWS, 2 * OUT_DIM], _bf16,
                           kind="ExternalInput") for q in range(NT)]
    gidx = nc.dram_tensor("gidx", [P, meta["tot_idx_cols"]], _i16,
                          kind="ExternalInput")
    ldst = nc.dram_tensor("ldst", [P, meta["tot_chunks"]], _bf16,
                          kind="ExternalInput")
    max_gch = meta["max_gch"]
    ideg = nc.dram_tensor("ideg", [P, NG], _f32, kind="ExternalInput")
    brep = nc.dram_tensor("brep", [P, OUT_DIM], _f32, kind="ExternalInput")
    # iotar[p, m, c] = m  (bf16) — constant compare target for one-hot builds
    iotar = nc.dram_tensor("iotar", [P, P, max_gch], _bf16,
                           kind="ExternalInput")
    # partition-major output: out[p, g, f] = result(g*128+p, f)
    out = nc.dram_tensor("out", [P, NG, OUT_DIM], _f32, kind="ExternalOutput")

    with tile.TileContext(nc) as tc:
        loop = tc.For_i(0, repeat, 1) if repeat > 1 \
            else contextlib.nullcontext()
        with loop, \
                tc.tile_pool(name="const", bufs=1) as cpool, \
                tc.tile_pool(name="gath", bufs=3) as gpool, \
                tc.tile_pool(name="meta", bufs=3) as mpool, \
                tc.tile_pool(name="onehot", bufs=2) as opool, \
                tc.tile_pool(name="epi", bufs=3) as epool, \
                tc.tile_pool(name="psum", bufs=8, space="PSUM") as psum:
            bt = cpool.tile([P, OUT_DIM], _f32, tag="b")
            it = cpool.tile([P, P, max_gch], _bf16, tag="iotar")
            nc.sync.dma_start(out=bt[:], in_=brep[:, :])
            nc.sync.dma_start(out=it[:], in_=iotar[:, :, :])

            dt_ = cpool.tile([P, NG], _f32, tag="deg")
            norm = cpool.tile([P, NG], _f32, tag="norm")
            nc.sync.dma_start(out=dt_[:], in_=ideg[:, :])
            nc.vector.tensor_scalar_max(out=dt_[:], in0=dt_[:], scalar1=1.0)
            nc.vector.reciprocal(out=dt_[:], in_=dt_[:])
            nc.scalar.sqrt(out=norm[:], in_=dt_[:])

            # persistent per-group staging: y = x - max(x); s = sum(exp)
            y_all = cpool.tile([P, NG, OUT_DIM], _f32, tag="yall")
            s_all = cpool.tile([P, NG], _f32, tag="sall")

            for ri, rnd in enumerate(meta["rounds"]):
                gs = rnd["groups"]
                rg = len(gs)
                nch = rnd["nch"]
                nidx_cols = sum(rnd["q_numidx"]) // 16
                ixt = mpool.tile([P, nidx_cols], _i16, tag="ix")
                nc.sync.dma_start(
                    out=ixt[:],
                    in_=gidx[:, rnd["idx_off"]:rnd["idx_off"] + nidx_cols])
                ldt = mpool.tile([P, nch], _bf16, tag="ld")
                nc.scalar.dma_start(
                    out=ldt[:],
                    in_=ldst[:, rnd["ch_off"]:rnd["ch_off"] + nch])

                gt = gpool.tile([P, nch, OUT_DIM], _bf16, tag="gt")
                icol = 0
                for q in range(NT):
                    for par in range(2):
                        nq = rnd["q_numidx"][q * 2 + par]
                        if nq == 0:
                            continue
                        co = rnd["q_choff"][q * 2 + par]
                        _dma_gather_half(
                            nc.gpsimd,
                            out_ap=gt[:, co:co + nq // P, :],
                            in_ap=tabs[q][:, par * OUT_DIM:(par + 1) * OUT_DIM],
                            idxs_ap=ixt[:, icol:icol + nq // 16],
                            num_idxs=nq,
                            elem_size=OUT_DIM,
                            elem_step=2 * OUT_DIM,
                            single_packet=False,
                        )
                        icol += nq // 16

                xr = epool.tile([P, ROUND_G, OUT_DIM], _f32, tag="xr")
                for i, g in enumerate(gs):
                    ldt0, ngch = rnd["gldt"][g]
                    gtcols = rnd["ggt"][g]
                    # one-hot, chunk-last: oh[k, m, c] = (ldst[k,col_c]==m)
                    # all APs keep a packed last dim -> DVE 2x mode
                    oh = opool.tile([P, P, max_gch], _bf16, tag="oh")
                    nc.vector.tensor_tensor(
                        out=oh[:, :, 0:ngch],
                        in0=_expand_mid(ldt[:, ldt0:ldt0 + ngch], P),
                        in1=it[:, :, 0:ngch],
                        op=mybir.AluOpType.is_equal)
                    acc = psum.tile([P, OUT_DIM], _f32, tag="acc")
                    for k, cg in enumerate(gtcols):
                        nc.tensor.matmul(
                            acc[:], oh[:, :, k], gt[:, cg, :],
                            start=(k == 0), stop=(k == ngch - 1))
                    nc.scalar.activation(
                        out=xr[:, i, :], in_=acc[:],
                        func=mybir.ActivationFunctionType.Identity,
                        scale=norm[:, g:g + 1])

                g0 = gs[0]
                # batched epilogue for the round's rg groups
                nc.vector.tensor_tensor(
                    out=xr[:, :rg, :], in0=xr[:, :rg, :],
                    in1=_expand_mid(bt[:, :], rg),
                    op=mybir.AluOpType.add)
                nmx = epool.tile([P, ROUND_G], _f32, tag="nmx")
                nc.vector.tensor_reduce(out=nmx[:, :rg], in_=xr[:, :rg, :],
                                        axis=mybir.AxisListType.X,
                                        op=mybir.AluOpType.max,
                                        negate=True)
                nc.vector.tensor_tensor(
                    out=y_all[:, g0:g0 + rg, :], in0=xr[:, :rg, :],
                    in1=_expand_last(nmx[:, :rg], OUT_DIM),
                    op=mybir.AluOpType.add)
                e = epool.tile([P, ROUND_G, OUT_DIM], _f32, tag="e")
                for i, g in enumerate(gs):
                    nc.scalar.activation(
                        out=e[:, i, :], in_=y_all[:, g, :],
                        func=mybir.ActivationFunctionType.Exp,
                        accum_out=s_all[:, g:g + 1])

            # single Ln over all groups, then finalize + store per round
            ls_all = cpool.tile([P, NG], _f32, tag="lsall")
            nc.scalar.activation(out=ls_all[:], in_=s_all[:],
                                 func=mybir.ActivationFunctionType.Ln)
            for rnd in meta["rounds"]:
                gs = rnd["groups"]
                g0 = gs[0]
                rg = len(gs)
                fin = epool.tile([P, ROUND_G, OUT_DIM], _f32, tag="fin")
                nc.vector.tensor_tensor(
                    out=fin[:, :rg, :], in0=y_all[:, g0:g0 + rg, :],
                    in1=_expand_last(ls_all[:, g0:g0 + rg], OUT_DIM),
                    op=mybir.AluOpType.subtract)
                nc.scalar.dma_start(out=out[:, g0:g0 + rg, :],
                                    in_=fin[:, :rg, :])
    nc.compile()
    return nc


# ------------------------------------------------------------- host prep
def _wrap_idx16(flat):
    """int16 index list -> [128, len/16] wrapped layout (16-partition groups,
    replicated across the 8 gpsimd cores)."""
    n = len(flat)
    s = n // 16
    arr = np.empty((P, s), dtype=np.int16)
    blk = flat.reshape(s, 16).T  # [16, s]
    for grp in range(8):
        arr[grp * 16:(grp + 1) * 16, :] = blk
    return arr


def prepare(h, W, b, edges):
    h = np.asarray(h, dtype=np.float32)
    W = np.asarray(W, dtype=np.float32)
    b = np.asarray(b, dtype=np.float32)
    src = np.asarray(edges[0], dtype=np.int64)
    dst = np.asarray(edges[1], dtype=np.int64)

    out_deg = np.bincount(src, minlength=N_NODES).astype(np.float32)
    in_deg = np.bincount(dst, minlength=N_NODES).astype(np.float32)

    # global m-table row for each src node (padded per-core layout), then
    # paired-row coordinates: pair index + parity -> (sub-table, class)
    score = src // G
    mrow = score * GPAD + (src - score * G)
    pair = mrow >> 1
    par = mrow & 1
    qtab = pair // TROWS
    lrow = (pair - qtab * TROWS).astype(np.int16)
    cls = qtab * 2 + par

    dcore = dst // G
    dloc = dst - dcore * G
    grp = dloc // P
    ldst_v = (dloc - grp * P).astype(np.float32)

    # bucket = (dst-core, group, class)
    bucket = (dcore * NG + grp) * NCLS + cls
    order = np.argsort(bucket, kind="stable")
    bucket_s = bucket[order]
    lrow_s = lrow[order]
    ldst_s = ldst_v[order]

    nbuck = NCORES * NG * NCLS
    counts = np.bincount(bucket_s, minlength=nbuck).reshape(NCORES, NG, NCLS)
    starts = np.zeros(nbuck + 1, dtype=np.int64)
    np.cumsum(counts.reshape(-1), out=starts[1:])

    # uniform capacity per (group, class): max over cores, ceil to 128
    cap = counts.max(axis=0)                      # [NG, NCLS]
    cap128 = ((cap + P - 1) // P) * P             # [NG, NCLS]

    # round structure (uniform across cores)
    # gather tile columns: (class, g, chunk) order; ldst: (g, class, chunk)
    rounds = []
    idx_off = 0
    ch_off = 0
    for r0 in range(0, NG, ROUND_G):
        gs = list(range(r0, min(r0 + ROUND_G, NG)))
        q_numidx, q_choff = [], []
        gt_col = {}          # (g, c) -> gather-tile column base (within round)
        cursor = 0
        for c in range(NCLS):
            q_choff.append(cursor)
            tot = 0
            for g in gs:
                cp = int(cap128[g, c])
                gt_col[(g, c)] = cursor
                cursor += cp // P
                tot += cp
            q_numidx.append(tot)
        gldt, ggt = {}, {}
        lcur = 0
        for g in gs:
            cols = []
            for c in range(NCLS):
                cols.extend(range(gt_col[(g, c)],
                                  gt_col[(g, c)] + int(cap128[g, c]) // P))
            gldt[g] = (lcur, len(cols))
            ggt[g] = cols
            lcur += len(cols)
        rounds.append(dict(groups=gs, q_numidx=q_numidx, q_choff=q_choff,
                           nch=cursor, idx_off=idx_off, ch_off=ch_off,
                           gldt=gldt, ggt=ggt, gt_col=gt_col))
        idx_off += sum(q_numidx) // 16
        ch_off += cursor
    max_gch = max(rnd["gldt"][g][1] for rnd in rounds for g in rnd["groups"])
    meta = dict(rounds=rounds, tot_idx_cols=idx_off, tot_chunks=ch_off,
                max_gch=max_gch)

    # per-core gidx / ldst arrays
    gidx_cores = []
    ldst_cores = []
    for c0 in range(NCORES):
        flat_idx = np.zeros(idx_off * 16, dtype=np.int16)
        ld = np.full((P, ch_off), PAD_LDST, dtype=np.float32)
        for rnd in rounds:
            pos = rnd["idx_off"] * 16
            for c in range(NCLS):
                for g in rnd["groups"]:
                    bid = (c0 * NG + g) * NCLS + c
                    s0, s1 = starts[bid], starts[bid + 1]
                    n = s1 - s0
                    capq = int(cap128[g, c])
                    flat_idx[pos:pos + n] = lrow_s[s0:s1]
                    pos += capq
                    # ldst column base: group-major layout
                    qch0 = sum(int(cap128[g, c2]) // P for c2 in range(c))
                    base = rnd["ch_off"] + rnd["gldt"][g][0] + qch0
                    j = np.arange(n)
                    ld[j % P, base + j // P] = ldst_s[s0:s1]
        gidx_cores.append(_wrap_idx16(flat_idx))
        ldst_cores.append(ld.astype(ml_dtypes.bfloat16))

    # degree tiles [128, NG] (partition = node % 128 within group)
    def deg_tile(deg):
        tiles = []
        for c in range(NCORES):
            d = np.ones(GPAD, dtype=np.float32)
            d[:G] = deg[c * G:(c + 1) * G]
            tiles.append(d.reshape(NG, P).T.copy())
        return tiles

    odeg_tiles = deg_tile(out_deg)
    ideg_tiles = deg_tile(in_deg)

    hT_cores = []
    for c in range(NCORES):
        hp = np.zeros((GPAD, IN_DIM), dtype=np.float32)
        hp[:G] = h[c * G:(c + 1) * G]
        # [2, 128, GPAD]: k-halves, contiguous along nodes for wide DMAs
        ht = np.ascontiguousarray(hp.T.reshape(2, P, GPAD))
        hT_cores.append(ht.astype(ml_dtypes.bfloat16))

    wt = np.ascontiguousarray(W.reshape(2, P, OUT_DIM)).astype(
        ml_dtypes.bfloat16)
    brep = np.broadcast_to(b, (P, OUT_DIM)).copy()
    # iotar[p, m, c] = m
    iotar = np.broadcast_to(
        np.arange(P, dtype=np.float32)[None, :, None],
        (P, P, max_gch)).astype(ml_dtypes.bfloat16).copy()

    return dict(meta=meta, gidx=gidx_cores, ldst=ldst_cores,
                odeg=odeg_tiles, ideg=ideg_tiles, hT=hT_cores,
                W=wt, brep=brep, iotar=iotar)


_cache = {}


def _get_programs(meta):
    if "a" not in _cache:
        _cache["a"] = build_launch_a()
    if "b" not in _cache:
        _cache["b"] = build_launch_b(meta)
    return _cache["a"], _cache["b"]


def run_launch_a(nc_a, prep):
    in_maps = [{"hT": prep["hT"][c], "W": prep["W"], "odeg": prep["odeg"][c]}
               for c in range(NCORES)]
    res = run_bass_kernel_spmd(nc_a, in_maps, list(range(NCORES)))
    # m[p, g*64+f] -> rows (g*128+p, f)
    shards = []
    for r in res.results:
        md = np.asarray(r["m"]).reshape(P, NG, OUT_DIM)
        shards.append(md.transpose(1, 0, 2).reshape(GPAD, OUT_DIM))
    return shards


def run_launch_b(nc_b, prep, m_shards):
    m_full = np.concatenate(m_shards, axis=0)  # [NCORES*GPAD, 64] bf16
    mp = m_full.reshape(NPAIR, 2 * OUT_DIM)    # paired rows, 256 B stride
    tabs = {f"t{q}": np.ascontiguousarray(mp[q * TROWS:(q + 1) * TROWS])
            for q in range(NT)}
    in_maps = [dict(tabs, gidx=prep["gidx"][c], ldst=prep["ldst"][c],
                    ideg=prep["ideg"][c], brep=prep["brep"],
                    iotar=prep["iotar"]) for c in range(NCORES)]
    res = run_bass_kernel_spmd(nc_b, in_maps, list(range(NCORES)))
    outs = []
    for r in res.results:
        od = np.asarray(r["out"])  # [P, NG, 64]
        outs.append(od.transpose(1, 0, 2).reshape(GPAD, OUT_DIM)[:G])
    return np.concatenate(outs, axis=0)


def kernel(h, W, b, edges):
    prep = prepare(h, W, b, edges)
    nc_a, nc_b = _get_programs(prep["meta"])
    m_shards = run_launch_a(nc_a, prep)
    out = run_launch_b(nc_b, prep, m_shards)
    return out.astype(np.float32)


# revision 12
# speedup vs baseline: 2.1388x; 1.0047x over previous
"""GraphConv (DGL norm='both') + log_softmax on 8 Trainium2 NeuronCores.

Strategy (per sharding hint): partition nodes across the 8 cores by range.
  Launch A (per core): project its 12500-node slice m = (h @ W) * out_deg^-1/2
  in bf16 (PE bf16, PSUM f32 accumulate).
  Host: concatenate the 8 projected shards into a replicated gather table,
  viewed as PAIRED rows [50176, 128] bf16 so the table row stride is 256 B
  (DMA descriptor encoding granularity) while each gather moves only the
  needed 128-B half-row (the pair parity selects a 64-col offset).
  Launch B (per core): for its 12500 dst nodes, gather m[src] half-rows for
  all in-edges (dma_gather, edges pre-sorted by dst group), segment-sum via
  one-hot matmuls accumulating in PSUM, then norm/bias/log_softmax.

Degrees and the sorted/padded edge metadata are sharding-prep computed on the
host (numpy); all FLOPs on h/W/b/m (projection, normalization, aggregation,
softmax) run on device.
"""

import contextlib

import numpy as np
import ml_dtypes

import concourse.bass as bass
import concourse.bacc as bacc
import concourse.mybir as mybir
import concourse.tile as tile
from concourse.bass import AP
from concourse.bass_utils import run_bass_kernel_spmd

P = 128
N_NODES = 100000
N_EDGES = 3200000
IN_DIM = 256
OUT_DIM = 64
NCORES = 8
G = N_NODES // NCORES            # 12500 nodes per core
NG = (G + P - 1) // P            # 98 groups of 128 dst nodes (last has 84)
GPAD = NG * P                    # 12544
NPAIR = (NCORES * GPAD) // 2     # 50176 paired table rows
NT = 2                           # sub-tables (int16 index limit)
TROWS = NPAIR // NT              # 25088 rows per sub-table
NCLS = NT * 2                    # gather classes: (sub-table, parity)
ROUND_G = 8                      # dst groups per gather round
HBLK = 16                        # dst groups per hT load in launch A
PAD_LDST = 200.0                 # local-dst for padded edges (>127, exact bf16)

_f32 = mybir.dt.float32
_bf16 = mybir.dt.bfloat16
_i16 = mybir.dt.int16


def _expand_mid(ap, n):
    """[P, C] AP -> [P, n, C] AP repeating each partition row n times
    (middle broadcast keeps the last dim packed, so DVE 2x mode applies)."""
    (ps, pc), (cs, cc) = ap.ap[0], ap.ap[1]
    return AP(ap.tensor, ap.offset, [[ps, pc], [0, n], [cs, cc]])


def _expand_last(ap, n):
    """[P, C] AP -> [P, C, n] AP repeating each element n times along a new
    innermost (stride-0) dim."""
    (ps, pc), (cs, cc) = ap.ap[0], ap.ap[1]
    return AP(ap.tensor, ap.offset, [[ps, pc], [cs, cc], [0, n]])


def _dma_gather_half(eng, out_ap, in_ap, idxs_ap, num_idxs, elem_size,
                     elem_step, single_packet=False):
    """dma_gather with a sub-256B payload (row stride must stay 256B-aligned:
    elem_step * dtype_size % 256 == 0). Same IR as bass's dma_gather helper,
    minus its payload-granularity assert (the HW descriptor only constrains
    the stride; the payload is free-form)."""
    stride_bytes = elem_step * mybir.dt.size(in_ap.dtype)
    assert stride_bytes % 256 == 0
    assert in_ap.ap[0][0] == elem_step
    return eng.add_instruction(
        mybir.InstDMAGatherAnt(
            name=eng.bass.get_next_instruction_name(),
            ins=[*eng.lower_ap_dma(in_ap, for_custom_bir_dma=True),
                 eng.lower_ap(idxs_ap),
                 eng.lower_val_access(eng.to_reg(num_idxs))],
            outs=[eng.lower_ap(out_ap)],
            transpose=False,
            num_idxs=num_idxs,
            elem_size=elem_size,
            stride_bytes_256=stride_bytes // 256,
            gen_mode=0,
            single_packet=single_packet,
            queue_num=0,
            sbuf_tokens_per_rank=0,
            sbuf_free_dim_per_rank=0,
            sbuf_free_dim_pad_per_rank=0,
            sbuf_byte_offset=0,
        ))


# ---------------------------------------------------------------- launch A
def build_launch_a(repeat=1):
    nc = bacc.Bacc("TRN2", target_bir_lowering=False, debug=False,
                   num_devices=NCORES)
    hT = nc.dram_tensor("hT", [2, P, GPAD], _bf16, kind="ExternalInput")
    W = nc.dram_tensor("W", [2, P, OUT_DIM], _bf16, kind="ExternalInput")
    odeg = nc.dram_tensor("odeg", [P, NG], _f32, kind="ExternalInput")
    # partition-major projected features: m[p, g*64+f] = m_row(g*128+p, f)
    m = nc.dram_tensor("m", [P, NG * OUT_DIM], _bf16, kind="ExternalOutput")

    with tile.TileContext(nc) as tc:
        loop = tc.For_i(0, repeat, 1) if repeat > 1 \
            else contextlib.nullcontext()
        with loop, \
                tc.tile_pool(name="const", bufs=1) as cpool, \
                tc.tile_pool(name="hblk", bufs=3) as hpool, \
                tc.tile_pool(name="mstage", bufs=2) as mpool, \
                tc.tile_pool(name="psum", bufs=8, space="PSUM") as psum:
            w0 = cpool.tile([P, OUT_DIM], _bf16, tag="w0")
            w1 = cpool.tile([P, OUT_DIM], _bf16, tag="w1")
            nc.sync.dma_start(out=w0[:], in_=W[0, :, :])
            nc.sync.dma_start(out=w1[:], in_=W[1, :, :])

            dt_ = cpool.tile([P, NG], _f32, tag="deg")
            norm = cpool.tile([P, NG], _f32, tag="norm")
            nc.sync.dma_start(out=dt_[:], in_=odeg[:, :])
            nc.vector.tensor_scalar_max(out=dt_[:], in0=dt_[:], scalar1=1.0)
            nc.vector.reciprocal(out=dt_[:], in_=dt_[:])
            nc.scalar.sqrt(out=norm[:], in_=dt_[:])

            for g0 in range(0, NG, HBLK):
                nb = min(HBLK, NG - g0)
                l0 = hpool.tile([P, HBLK * P], _bf16, tag="l0")
                l1 = hpool.tile([P, HBLK * P], _bf16, tag="l1")
                nc.sync.dma_start(out=l0[:, :nb * P],
                                  in_=hT[0, :, g0 * P:(g0 + nb) * P])
                nc.scalar.dma_start(out=l1[:, :nb * P],
                                    in_=hT[1, :, g0 * P:(g0 + nb) * P])
                ms = mpool.tile([P, HBLK, OUT_DIM], _bf16, tag="ms")
                # 8 groups per PSUM bank; norm scaling batched on DVE
                for j0 in range(0, nb, 8):
                    nj = min(8, nb - j0)
                    acc8 = psum.tile([P, 8, OUT_DIM], _f32, tag="acc8")
                    for j in range(j0, j0 + nj):
                        nc.tensor.matmul(acc8[:, j - j0, :],
                                         l0[:, j * P:(j + 1) * P], w0[:],
                                         start=True, stop=False)
                        nc.tensor.matmul(acc8[:, j - j0, :],
                                         l1[:, j * P:(j + 1) * P], w1[:],
                                         start=False, stop=True)
                    nc.vector.tensor_tensor(
                        out=ms[:, j0:j0 + nj, :], in0=acc8[:, :nj, :],
                        in1=_expand_last(norm[:, g0 + j0:g0 + j0 + nj],
                                         OUT_DIM),
                        op=mybir.AluOpType.mult)
                nc.gpsimd.dma_start(
                    out=m[:, g0 * OUT_DIM:(g0 + nb) * OUT_DIM],
                    in_=ms[:, :nb, :])
    nc.compile()
    return nc


# ---------------------------------------------------------------- launch B
def build_launch_b(meta, repeat=1):
    """meta["rounds"][i]:
      groups; q_numidx[NCLS]; q_choff[NCLS]; nch; idx_off; ch_off
      gldt: {g: (ldt_col_start, ngch)}   # ldst cols, group-major contiguous
      ggt:  {g: [gt_column, ...]}        # gather-tile column per oh chunk
    """
    nc = bacc.Bacc("TRN2", target_bir_lowering=False, debug=False,
                   num_devices=NCORES)
    tabs = [nc.dram_tensor(f"t{q}", [TROWS, 2 * OUT_DIM], _bf16,
                           kind="ExternalInput") for q in range(NT)]
    gidx = nc.dram_tensor("gidx", [P, meta["tot_idx_cols"]], _i16,
                          kind="ExternalInput")
    ldst = nc.dram_tensor("ldst", [P, meta["tot_chunks"]], _bf16,
                          kind="ExternalInput")
    max_gch = meta["max_gch"]
    ideg = nc.dram_tensor("ideg", [P, NG], _f32, kind="ExternalInput")
    brep = nc.dram_tensor("brep", [P, OUT_DIM], _f32, kind="ExternalInput")
    # iotar[p, m, c] = m  (bf16) — constant compare target for one-hot builds
    iotar = nc.dram_tensor("iotar", [P, P, max_gch], _bf16,
                           kind="ExternalInput")
    # partition-major output: out[p, g, f] = result(g*128+p, f)
    out = nc.dram_tensor("out", [P, NG, OUT_DIM], _f32, kind="ExternalOutput")

    with tile.TileContext(nc) as tc:
        loop = tc.For_i(0, repeat, 1) if repeat > 1 \
            else contextlib.nullcontext()
        with loop, \
                tc.tile_pool(name="const", bufs=1) as cpool, \
                tc.tile_pool(name="gath", bufs=2) as gpool, \
                tc.tile_pool(name="meta", bufs=3) as mpool, \
                tc.tile_pool(name="onehot", bufs=4) as opool, \
                tc.tile_pool(name="epi", bufs=3) as epool, \
                tc.tile_pool(name="psum", bufs=8, space="PSUM") as psum:
            bt = cpool.tile([P, OUT_DIM], _f32, tag="b")
            it = cpool.tile([P, P, max_gch], _bf16, tag="iotar")
            nc.sync.dma_start(out=bt[:], in_=brep[:, :])
            nc.sync.dma_start(out=it[:], in_=iotar[:, :, :])

            dt_ = cpool.tile([P, NG], _f32, tag="deg")
            norm = cpool.tile([P, NG], _f32, tag="norm")
            nc.sync.dma_start(out=dt_[:], in_=ideg[:, :])
            nc.vector.tensor_scalar_max(out=dt_[:], in0=dt_[:], scalar1=1.0)
            nc.vector.reciprocal(out=dt_[:], in_=dt_[:])
            nc.scalar.sqrt(out=norm[:], in_=dt_[:])

            s_all = cpool.tile([P, NG], _f32, tag="sall")
            ls_all = cpool.tile([P, NG], _f32, tag="lsall")

            for ri, rnd in enumerate(meta["rounds"]):
                gs = rnd["groups"]
                rg = len(gs)
                nch = rnd["nch"]
                nidx_cols = sum(rnd["q_numidx"]) // 16
                ixt = mpool.tile([P, nidx_cols], _i16, tag="ix")
                nc.sync.dma_start(
                    out=ixt[:],
                    in_=gidx[:, rnd["idx_off"]:rnd["idx_off"] + nidx_cols])
                ldt = mpool.tile([P, nch], _bf16, tag="ld")
                nc.scalar.dma_start(
                    out=ldt[:],
                    in_=ldst[:, rnd["ch_off"]:rnd["ch_off"] + nch])

                gt = gpool.tile([P, nch, OUT_DIM], _bf16, tag="gt")
                icol = 0
                for q in range(NT):
                    for par in range(2):
                        nq = rnd["q_numidx"][q * 2 + par]
                        if nq == 0:
                            continue
                        co = rnd["q_choff"][q * 2 + par]
                        _dma_gather_half(
                            nc.gpsimd,
                            out_ap=gt[:, co:co + nq // P, :],
                            in_ap=tabs[q][:, par * OUT_DIM:(par + 1) * OUT_DIM],
                            idxs_ap=ixt[:, icol:icol + nq // 16],
                            num_idxs=nq,
                            elem_size=OUT_DIM,
                            elem_step=2 * OUT_DIM,
                            single_packet=False,
                        )
                        icol += nq // 16

                xr = epool.tile([P, ROUND_G, OUT_DIM], _f32, tag="xr")
                for i, g in enumerate(gs):
                    ldt0, ngch = rnd["gldt"][g]
                    gtcols = rnd["ggt"][g]
                    # one-hot, chunk-last: oh[k, m, c] = (ldst[k,col_c]==m)
                    # all APs keep a packed last dim -> DVE 2x mode
                    oh = opool.tile([P, P, max_gch], _bf16, tag="oh")
                    nc.vector.tensor_tensor(
                        out=oh[:, :, 0:ngch],
                        in0=_expand_mid(ldt[:, ldt0:ldt0 + ngch], P),
                        in1=it[:, :, 0:ngch],
                        op=mybir.AluOpType.is_equal)
                    acc = psum.tile([P, OUT_DIM], _f32, tag="acc")
                    for k, cg in enumerate(gtcols):
                        nc.tensor.matmul(
                            acc[:], oh[:, :, k], gt[:, cg, :],
                            start=(k == 0), stop=(k == ngch - 1))
                    nc.scalar.activation(
                        out=xr[:, i, :], in_=acc[:],
                        func=mybir.ActivationFunctionType.Identity,
                        scale=norm[:, g:g + 1])

                g0 = gs[0]
                # batched epilogue for the round's rg groups
                nc.vector.tensor_tensor(
                    out=xr[:, :rg, :], in0=xr[:, :rg, :],
                    in1=_expand_mid(bt[:, :], rg),
                    op=mybir.AluOpType.add)
                nmx = epool.tile([P, ROUND_G], _f32, tag="nmx")
                nc.vector.tensor_reduce(out=nmx[:, :rg], in_=xr[:, :rg, :],
                                        axis=mybir.AxisListType.X,
                                        op=mybir.AluOpType.max,
                                        negate=True)
                yr = epool.tile([P, ROUND_G, OUT_DIM], _f32, tag="yr")
                nc.vector.tensor_tensor(
                    out=yr[:, :rg, :], in0=xr[:, :rg, :],
                    in1=_expand_last(nmx[:, :rg], OUT_DIM),
                    op=mybir.AluOpType.add)
                e = epool.tile([P, ROUND_G, OUT_DIM], _f32, tag="e")
                for i, g in enumerate(gs):
                    nc.scalar.activation(
                        out=e[:, i, :], in_=yr[:, i, :],
                        func=mybir.ActivationFunctionType.Exp,
                        accum_out=s_all[:, g:g + 1])
                # per-round log_softmax finalize (Ln is per-group)
                nc.scalar.activation(
                    out=ls_all[:, g0:g0 + rg], in_=s_all[:, g0:g0 + rg],
                    func=mybir.ActivationFunctionType.Ln)
                fin = epool.tile([P, ROUND_G, OUT_DIM], _f32, tag="fin")
                nc.vector.tensor_tensor(
                    out=fin[:, :rg, :], in0=yr[:, :rg, :],
                    in1=_expand_last(ls_all[:, g0:g0 + rg], OUT_DIM),
                    op=mybir.AluOpType.subtract)
                nc.scalar.dma_start(out=out[:, g0:g0 + rg, :],
                                    in_=fin[:, :rg, :])
    nc.compile()
    return nc


# ------------------------------------------------------------- host prep
def _wrap_idx16(flat):
    """int16 index list -> [128, len/16] wrapped layout (16-partition groups,
    replicated across the 8 gpsimd cores)."""
    n = len(flat)
    s = n // 16
    arr = np.empty((P, s), dtype=np.int16)
    blk = flat.reshape(s, 16).T  # [16, s]
    for grp in range(8):
        arr[grp * 16:(grp + 1) * 16, :] = blk
    return arr


def prepare(h, W, b, edges):
    h = np.asarray(h, dtype=np.float32)
    W = np.asarray(W, dtype=np.float32)
    b = np.asarray(b, dtype=np.float32)
    src = np.asarray(edges[0], dtype=np.int64)
    dst = np.asarray(edges[1], dtype=np.int64)

    out_deg = np.bincount(src, minlength=N_NODES).astype(np.float32)
    in_deg = np.bincount(dst, minlength=N_NODES).astype(np.float32)

    # global m-table row for each src node (padded per-core layout), then
    # paired-row coordinates: pair index + parity -> (sub-table, class)
    score = src // G
    mrow = score * GPAD + (src - score * G)
    pair = mrow >> 1
    par = mrow & 1
    qtab = pair // TROWS
    lrow = (pair - qtab * TROWS).astype(np.int16)
    cls = qtab * 2 + par

    dcore = dst // G
    dloc = dst - dcore * G
    grp = dloc // P
    ldst_v = (dloc - grp * P).astype(np.float32)

    # bucket = (dst-core, group, class)
    bucket = (dcore * NG + grp) * NCLS + cls
    order = np.argsort(bucket, kind="stable")
    bucket_s = bucket[order]
    lrow_s = lrow[order]
    ldst_s = ldst_v[order]

    nbuck = NCORES * NG * NCLS
    counts = np.bincount(bucket_s, minlength=nbuck).reshape(NCORES, NG, NCLS)
    starts = np.zeros(nbuck + 1, dtype=np.int64)
    np.cumsum(counts.reshape(-1), out=starts[1:])

    # uniform capacity per (group, class): max over cores, ceil to 128
    cap = counts.max(axis=0)                      # [NG, NCLS]
    cap128 = ((cap + P - 1) // P) * P             # [NG, NCLS]

    # round structure (uniform across cores)
    # gather tile columns: (class, g, chunk) order; ldst: (g, class, chunk)
    rounds = []
    idx_off = 0
    ch_off = 0
    for r0 in range(0, NG, ROUND_G):
        gs = list(range(r0, min(r0 + ROUND_G, NG)))
        q_numidx, q_choff = [], []
        gt_col = {}          # (g, c) -> gather-tile column base (within round)
        cursor = 0
        for c in range(NCLS):
            q_choff.append(cursor)
            tot = 0
            for g in gs:
                cp = int(cap128[g, c])
                gt_col[(g, c)] = cursor
                cursor += cp // P
                tot += cp
            q_numidx.append(tot)
        gldt, ggt = {}, {}
        lcur = 0
        for g in gs:
            cols = []
            for c in range(NCLS):
                cols.extend(range(gt_col[(g, c)],
                                  gt_col[(g, c)] + int(cap128[g, c]) // P))
            gldt[g] = (lcur, len(cols))
            ggt[g] = cols
            lcur += len(cols)
        rounds.append(dict(groups=gs, q_numidx=q_numidx, q_choff=q_choff,
                           nch=cursor, idx_off=idx_off, ch_off=ch_off,
                           gldt=gldt, ggt=ggt, gt_col=gt_col))
        idx_off += sum(q_numidx) // 16
        ch_off += cursor
    max_gch = max(rnd["gldt"][g][1] for rnd in rounds for g in rnd["groups"])
    meta = dict(rounds=rounds, tot_idx_cols=idx_off, tot_chunks=ch_off,
                max_gch=max_gch)

    # per-core gidx / ldst arrays
    gidx_cores = []
    ldst_cores = []
    for c0 in range(NCORES):
        flat_idx = np.zeros(idx_off * 16, dtype=np.int16)
        ld = np.full((P, ch_off), PAD_LDST, dtype=np.float32)
        for rnd in rounds:
            pos = rnd["idx_off"] * 16
            for c in range(NCLS):
                for g in rnd["groups"]:
                    bid = (c0 * NG + g) * NCLS + c
                    s0, s1 = starts[bid], starts[bid + 1]
                    n = s1 - s0
                    capq = int(cap128[g, c])
                    flat_idx[pos:pos + n] = lrow_s[s0:s1]
                    pos += capq
                    # ldst column base: group-major layout
                    qch0 = sum(int(cap128[g, c2]) // P for c2 in range(c))
                    base = rnd["ch_off"] + rnd["gldt"][g][0] + qch0
                    j = np.arange(n)
                    ld[j % P, base + j // P] = ldst_s[s0:s1]
        gidx_cores.append(_wrap_idx16(flat_idx))
        ldst_cores.append(ld.astype(ml_dtypes.bfloat16))

    # degree tiles [128, NG] (partition = node % 128 within group)
    def deg_tile(deg):
        tiles = []
        for c in range(NCORES):
            d = np.ones(GPAD, dtype=np.float32)
            d[:G] = deg[c * G:(c + 1) * G]
            tiles.append(d.reshape(NG, P).T.copy())
        return tiles

    odeg_tiles = deg_tile(out_deg)
    ideg_tiles = deg_tile(in_deg)

    hT_cores = []
    for c in range(NCORES):
        hp = np.zeros((GPAD, IN_DIM), dtype=np.float32)
        hp[:G] = h[c * G:(c + 1) * G]
        # [2, 128, GPAD]: k-halves, contiguous along nodes for wide DMAs
        ht = np.ascontiguousarray(hp.T.reshape(2, P, GPAD))
        hT_cores.append(ht.astype(ml_dtypes.bfloat16))

    wt = np.ascontiguousarray(W.reshape(2, P, OUT_DIM)).astype(
        ml_dtypes.bfloat16)
    brep = np.broadcast_to(b, (P, OUT_DIM)).copy()
    # iotar[p, m, c] = m
    iotar = np.broadcast_to(
        np.arange(P, dtype=np.float32)[None, :, None],
        (P, P, max_gch)).astype(ml_dtypes.bfloat16).copy()

    return dict(meta=meta, gidx=gidx_cores, ldst=ldst_cores,
                odeg=odeg_tiles, ideg=ideg_tiles, hT=hT_cores,
                W=wt, brep=brep, iotar=iotar)


_cache = {}


def _get_programs(meta):
    if "a" not in _cache:
        _cache["a"] = build_launch_a()
    if "b" not in _cache:
        _cache["b"] = build_launch_b(meta)
    return _cache["a"], _cache["b"]


def run_launch_a(nc_a, prep):
    in_maps = [{"hT": prep["hT"][c], "W": prep["W"], "odeg": prep["odeg"][c]}
               for c in range(NCORES)]
    res = run_bass_kernel_spmd(nc_a, in_maps, list(range(NCORES)))
    # m[p, g*64+f] -> rows (g*128+p, f)
    shards = []
    for r in res.results:
        md = np.asarray(r["m"]).reshape(P, NG, OUT_DIM)
        shards.append(md.transpose(1, 0, 2).reshape(GPAD, OUT_DIM))
    return shards


def run_launch_b(nc_b, prep, m_shards):
    m_full = np.concatenate(m_shards, axis=0)  # [NCORES*GPAD, 64] bf16
    mp = m_full.reshape(NPAIR, 2 * OUT_DIM)    # paired rows, 256 B stride
    tabs = {f"t{q}": np.ascontiguousarray(mp[q * TROWS:(q + 1) * TROWS])
            for q in range(NT)}
    in_maps = [dict(tabs, gidx=prep["gidx"][c], ldst=prep["ldst"][c],
                    ideg=prep["ideg"][c], brep=prep["brep"],
                    iotar=prep["iotar"]) for c in range(NCORES)]
    res = run_bass_kernel_spmd(nc_b, in_maps, list(range(NCORES)))
    outs = []
    for r in res.results:
        od = np.asarray(r["out"])  # [P, NG, 64]
        outs.append(od.transpose(1, 0, 2).reshape(GPAD, OUT_DIM)[:G])
    return np.concatenate(outs, axis=0)


def kernel(h, W, b, edges):
    prep = prepare(h, W, b, edges)
    nc_a, nc_b = _get_programs(prep["meta"])
    m_shards = run_launch_a(nc_a, prep)
    out = run_launch_b(nc_b, prep, m_shards)
    return out.astype(np.float32)


# revision 16
# speedup vs baseline: 2.1634x; 1.0115x over previous
"""GraphConv (DGL norm='both') + log_softmax on 8 Trainium2 NeuronCores.

Strategy (per sharding hint): partition nodes across the 8 cores by range.
  Launch A (per core): project its 12500-node slice m = (h @ W) * out_deg^-1/2
  in bf16 (PE bf16, PSUM f32 accumulate).
  Host: concatenate the 8 projected shards into a replicated gather table,
  viewed as PAIRED rows [50176, 128] bf16 so the table row stride is 256 B
  (DMA descriptor encoding granularity) while each gather moves only the
  needed 128-B half-row (the pair parity selects a 64-col offset).
  Launch B (per core): for its 12500 dst nodes, gather m[src] half-rows for
  all in-edges (dma_gather, edges pre-sorted by dst group), segment-sum via
  one-hot matmuls accumulating in PSUM, then norm/bias/log_softmax.

Degrees and the sorted/padded edge metadata are sharding-prep computed on the
host (numpy); all FLOPs on h/W/b/m (projection, normalization, aggregation,
softmax) run on device.
"""

import contextlib

import numpy as np
import ml_dtypes

import concourse.bass as bass
import concourse.bacc as bacc
import concourse.mybir as mybir
import concourse.tile as tile
from concourse.bass import AP
from concourse.bass_utils import run_bass_kernel_spmd

P = 128
N_NODES = 100000
N_EDGES = 3200000
IN_DIM = 256
OUT_DIM = 64
NCORES = 8
G = N_NODES // NCORES            # 12500 nodes per core
NG = (G + P - 1) // P            # 98 groups of 128 dst nodes (last has 84)
GPAD = NG * P                    # 12544
NPAIR = (NCORES * GPAD) // 2     # 50176 paired table rows
NT = 2                           # sub-tables (int16 index limit)
TROWS = NPAIR // NT              # 25088 rows per sub-table
NCLS = NT * 2                    # gather classes: (sub-table, parity)
ROUND_G = 8                      # max dst groups per gather round
# tapered round sizes: short first round starts compute early; short last
# rounds keep the post-gather drain chain small
ROUND_SIZES = [4] + [8] * 11 + [3, 2, 1]
assert sum(ROUND_SIZES) == NG
HBLK = 16                        # dst groups per hT load in launch A
PAD_LDST = 200.0                 # local-dst for padded edges (>127, exact bf16)

_f32 = mybir.dt.float32
_bf16 = mybir.dt.bfloat16
_i16 = mybir.dt.int16


def _expand_mid(ap, n):
    """[P, C] AP -> [P, n, C] AP repeating each partition row n times
    (middle broadcast keeps the last dim packed, so DVE 2x mode applies)."""
    (ps, pc), (cs, cc) = ap.ap[0], ap.ap[1]
    return AP(ap.tensor, ap.offset, [[ps, pc], [0, n], [cs, cc]])


def _expand_last(ap, n):
    """[P, C] AP -> [P, C, n] AP repeating each element n times along a new
    innermost (stride-0) dim."""
    (ps, pc), (cs, cc) = ap.ap[0], ap.ap[1]
    return AP(ap.tensor, ap.offset, [[ps, pc], [cs, cc], [0, n]])


def _dma_gather_half(eng, out_ap, in_ap, idxs_ap, num_idxs, elem_size,
                     elem_step, single_packet=False):
    """dma_gather with a sub-256B payload (row stride must stay 256B-aligned:
    elem_step * dtype_size % 256 == 0). Same IR as bass's dma_gather helper,
    minus its payload-granularity assert (the HW descriptor only constrains
    the stride; the payload is free-form)."""
    stride_bytes = elem_step * mybir.dt.size(in_ap.dtype)
    assert stride_bytes % 256 == 0
    assert in_ap.ap[0][0] == elem_step
    return eng.add_instruction(
        mybir.InstDMAGatherAnt(
            name=eng.bass.get_next_instruction_name(),
            ins=[*eng.lower_ap_dma(in_ap, for_custom_bir_dma=True),
                 eng.lower_ap(idxs_ap),
                 eng.lower_val_access(eng.to_reg(num_idxs))],
            outs=[eng.lower_ap(out_ap)],
            transpose=False,
            num_idxs=num_idxs,
            elem_size=elem_size,
            stride_bytes_256=stride_bytes // 256,
            gen_mode=0,
            single_packet=single_packet,
            queue_num=0,
            sbuf_tokens_per_rank=0,
            sbuf_free_dim_per_rank=0,
            sbuf_free_dim_pad_per_rank=0,
            sbuf_byte_offset=0,
        ))


# ---------------------------------------------------------------- launch A
def build_launch_a(repeat=1):
    nc = bacc.Bacc("TRN2", target_bir_lowering=False, debug=False,
                   num_devices=NCORES)
    hT = nc.dram_tensor("hT", [2, P, GPAD], _bf16, kind="ExternalInput")
    W = nc.dram_tensor("W", [2, P, OUT_DIM], _bf16, kind="ExternalInput")
    odeg = nc.dram_tensor("odeg", [P, NG], _f32, kind="ExternalInput")
    # partition-major projected features: m[p, g*64+f] = m_row(g*128+p, f)
    m = nc.dram_tensor("m", [P, NG * OUT_DIM], _bf16, kind="ExternalOutput")

    with tile.TileContext(nc) as tc:
        loop = tc.For_i(0, repeat, 1) if repeat > 1 \
            else contextlib.nullcontext()
        with loop, \
                tc.tile_pool(name="const", bufs=1) as cpool, \
                tc.tile_pool(name="hblk", bufs=3) as hpool, \
                tc.tile_pool(name="mstage", bufs=2) as mpool, \
                tc.tile_pool(name="psum", bufs=8, space="PSUM") as psum:
            w0 = cpool.tile([P, OUT_DIM], _bf16, tag="w0")
            w1 = cpool.tile([P, OUT_DIM], _bf16, tag="w1")
            nc.sync.dma_start(out=w0[:], in_=W[0, :, :])
            nc.sync.dma_start(out=w1[:], in_=W[1, :, :])

            dt_ = cpool.tile([P, NG], _f32, tag="deg")
            norm = cpool.tile([P, NG], _f32, tag="norm")
            nc.sync.dma_start(out=dt_[:], in_=odeg[:, :])
            nc.vector.tensor_scalar_max(out=dt_[:], in0=dt_[:], scalar1=1.0)
            nc.vector.reciprocal(out=dt_[:], in_=dt_[:])
            nc.scalar.sqrt(out=norm[:], in_=dt_[:])

            for g0 in range(0, NG, HBLK):
                nb = min(HBLK, NG - g0)
                # both k-halves in one DMA: dram-side AP iterates (p, half,
                # node) to match the SBUF tile's (partition, half, node)
                lh = hpool.tile([P, 2, HBLK * P], _bf16, tag="lh")
                src = AP(hT[0, :, :].tensor, g0 * P,
                         [[GPAD, P], [P * GPAD, 2], [1, nb * P]])
                nc.sync.dma_start(out=lh[:, :, :nb * P], in_=src)
                ms = mpool.tile([P, HBLK, OUT_DIM], _bf16, tag="ms")
                # 8 groups per PSUM bank; norm scaling batched on DVE
                for j0 in range(0, nb, 8):
                    nj = min(8, nb - j0)
                    acc8 = psum.tile([P, 8, OUT_DIM], _f32, tag="acc8")
                    for j in range(j0, j0 + nj):
                        nc.tensor.matmul(acc8[:, j - j0, :],
                                         lh[:, 0, j * P:(j + 1) * P], w0[:],
                                         start=True, stop=False)
                        nc.tensor.matmul(acc8[:, j - j0, :],
                                         lh[:, 1, j * P:(j + 1) * P], w1[:],
                                         start=False, stop=True)
                    nc.vector.tensor_tensor(
                        out=ms[:, j0:j0 + nj, :], in0=acc8[:, :nj, :],
                        in1=_expand_last(norm[:, g0 + j0:g0 + j0 + nj],
                                         OUT_DIM),
                        op=mybir.AluOpType.mult)
                nc.gpsimd.dma_start(
                    out=m[:, g0 * OUT_DIM:(g0 + nb) * OUT_DIM],
                    in_=ms[:, :nb, :])
    nc.compile()
    return nc


# ---------------------------------------------------------------- launch B
def build_launch_b(meta, repeat=1):
    """meta["rounds"][i]:
      groups; q_numidx[NCLS]; q_choff[NCLS]; nch; idx_off; ch_off
      gldt: {g: (ldt_col_start, ngch)}   # ldst cols, group-major contiguous
      ggt:  {g: [gt_column, ...]}        # gather-tile column per oh chunk
    """
    nc = bacc.Bacc("TRN2", target_bir_lowering=False, debug=False,
                   num_devices=NCORES)
    tabs = [nc.dram_tensor(f"t{q}", [TROWS, 2 * OUT_DIM], _bf16,
                           kind="ExternalInput") for q in range(NT)]
    gidx = nc.dram_tensor("gidx", [P, meta["tot_idx_cols"]], _i16,
                          kind="ExternalInput")
    ldst = nc.dram_tensor("ldst", [P, meta["tot_chunks"]], _bf16,
                          kind="ExternalInput")
    max_gch = meta["max_gch"]
    ideg = nc.dram_tensor("ideg", [P, NG], _f32, kind="ExternalInput")
    brep = nc.dram_tensor("brep", [P, OUT_DIM], _f32, kind="ExternalInput")
    # iotar[p, m, c] = m  (bf16) — constant compare target for one-hot builds
    iotar = nc.dram_tensor("iotar", [P, P, max_gch], _bf16,
                           kind="ExternalInput")
    # partition-major output: out[p, g, f] = result(g*128+p, f)
    out = nc.dram_tensor("out", [P, NG, OUT_DIM], _f32, kind="ExternalOutput")

    with tile.TileContext(nc) as tc:
        loop = tc.For_i(0, repeat, 1) if repeat > 1 \
            else contextlib.nullcontext()
        with loop, \
                tc.tile_pool(name="const", bufs=1) as cpool, \
                tc.tile_pool(name="gath", bufs=2) as gpool, \
                tc.tile_pool(name="meta", bufs=3) as mpool, \
                tc.tile_pool(name="onehot", bufs=4) as opool, \
                tc.tile_pool(name="epi", bufs=3) as epool, \
                tc.tile_pool(name="psum", bufs=8, space="PSUM") as psum:
            bt = cpool.tile([P, OUT_DIM], _f32, tag="b")
            it = cpool.tile([P, P, max_gch], _bf16, tag="iotar")
            nc.sync.dma_start(out=bt[:], in_=brep[:, :])
            nc.sync.dma_start(out=it[:], in_=iotar[:, :, :])

            dt_ = cpool.tile([P, NG], _f32, tag="deg")
            norm = cpool.tile([P, NG], _f32, tag="norm")
            nc.sync.dma_start(out=dt_[:], in_=ideg[:, :])
            nc.vector.tensor_scalar_max(out=dt_[:], in0=dt_[:], scalar1=1.0)
            nc.vector.reciprocal(out=dt_[:], in_=dt_[:])
            nc.scalar.sqrt(out=norm[:], in_=dt_[:])

            # pin the act table that holds Exp+Ln+Identity so the per-round
            # Ln never forces a table swap (the auto-placement pass would
            # pick exp-only first and reload 2x per round)
            from concourse.hw_specs import get_activation_tables
            want = {mybir.ActivationFunctionType.Exp,
                    mybir.ActivationFunctionType.Ln,
                    mybir.ActivationFunctionType.Identity}
            set_id = next(i for i, fs in
                          enumerate(get_activation_tables(nc.m.arch).values())
                          if want <= fs)
            nc.scalar.add_instruction(mybir.InstLoadActFuncSet(
                name=nc.scalar.bass.get_next_instruction_name(),
                ins=[], outs=[], act_func_set_id=set_id))

            s_all = cpool.tile([P, NG], _f32, tag="sall")
            ls_all = cpool.tile([P, NG], _f32, tag="lsall")

            for ri, rnd in enumerate(meta["rounds"]):
                gs = rnd["groups"]
                rg = len(gs)
                nch = rnd["nch"]
                nidx_cols = sum(rnd["q_numidx"]) // 16
                ixt = mpool.tile([P, nidx_cols], _i16, tag="ix")
                nc.sync.dma_start(
                    out=ixt[:],
                    in_=gidx[:, rnd["idx_off"]:rnd["idx_off"] + nidx_cols])
                ldt = mpool.tile([P, nch], _bf16, tag="ld")
                nc.scalar.dma_start(
                    out=ldt[:],
                    in_=ldst[:, rnd["ch_off"]:rnd["ch_off"] + nch])

                gt = gpool.tile([P, nch, OUT_DIM], _bf16, tag="gt")
                icol = 0
                for q in range(NT):
                    for par in range(2):
                        nq = rnd["q_numidx"][q * 2 + par]
                        if nq == 0:
                            continue
                        co = rnd["q_choff"][q * 2 + par]
                        _dma_gather_half(
                            nc.gpsimd,
                            out_ap=gt[:, co:co + nq // P, :],
                            in_ap=tabs[q][:, par * OUT_DIM:(par + 1) * OUT_DIM],
                            idxs_ap=ixt[:, icol:icol + nq // 16],
                            num_idxs=nq,
                            elem_size=OUT_DIM,
                            elem_step=2 * OUT_DIM,
                            single_packet=False,
                        )
                        icol += nq // 16

                xr = epool.tile([P, ROUND_G, OUT_DIM], _f32, tag="xr")
                for i, g in enumerate(gs):
                    ldt0, ngch = rnd["gldt"][g]
                    gtcols = rnd["ggt"][g]
                    # one-hot, chunk-last: oh[k, m, c] = (ldst[k,col_c]==m)
                    # all APs keep a packed last dim -> DVE 2x mode
                    oh = opool.tile([P, P, max_gch], _bf16, tag="oh")
                    nc.vector.tensor_tensor(
                        out=oh[:, :, 0:ngch],
                        in0=_expand_mid(ldt[:, ldt0:ldt0 + ngch], P),
                        in1=it[:, :, 0:ngch],
                        op=mybir.AluOpType.is_equal)
                    acc = psum.tile([P, OUT_DIM], _f32, tag="acc")
                    for k, cg in enumerate(gtcols):
                        nc.tensor.matmul(
                            acc[:], oh[:, :, k], gt[:, cg, :],
                            start=(k == 0), stop=(k == ngch - 1))
                    nc.scalar.activation(
                        out=xr[:, i, :], in_=acc[:],
                        func=mybir.ActivationFunctionType.Identity,
                        scale=norm[:, g:g + 1])

                g0 = gs[0]
                # batched epilogue for the round's rg groups
                nc.vector.tensor_tensor(
                    out=xr[:, :rg, :], in0=xr[:, :rg, :],
                    in1=_expand_mid(bt[:, :], rg),
                    op=mybir.AluOpType.add)
                nmx = epool.tile([P, ROUND_G], _f32, tag="nmx")
                nc.vector.tensor_reduce(out=nmx[:, :rg], in_=xr[:, :rg, :],
                                        axis=mybir.AxisListType.X,
                                        op=mybir.AluOpType.max,
                                        negate=True)
                yr = epool.tile([P, ROUND_G, OUT_DIM], _f32, tag="yr")
                nc.vector.tensor_tensor(
                    out=yr[:, :rg, :], in0=xr[:, :rg, :],
                    in1=_expand_last(nmx[:, :rg], OUT_DIM),
                    op=mybir.AluOpType.add)
                e = epool.tile([P, ROUND_G, OUT_DIM], _f32, tag="e")
                for i, g in enumerate(gs):
                    nc.scalar.activation(
                        out=e[:, i, :], in_=yr[:, i, :],
                        func=mybir.ActivationFunctionType.Exp,
                        accum_out=s_all[:, g:g + 1])
                # per-round log_softmax finalize (Ln is per-group)
                nc.scalar.activation(
                    out=ls_all[:, g0:g0 + rg], in_=s_all[:, g0:g0 + rg],
                    func=mybir.ActivationFunctionType.Ln)
                fin = epool.tile([P, ROUND_G, OUT_DIM], _f32, tag="fin")
                nc.vector.tensor_tensor(
                    out=fin[:, :rg, :], in0=yr[:, :rg, :],
                    in1=_expand_last(ls_all[:, g0:g0 + rg], OUT_DIM),
                    op=mybir.AluOpType.subtract)
                nc.scalar.dma_start(out=out[:, g0:g0 + rg, :],
                                    in_=fin[:, :rg, :])
    nc.compile()
    return nc


# ------------------------------------------------------------- host prep
def _wrap_idx16(flat):
    """int16 index list -> [128, len/16] wrapped layout (16-partition groups,
    replicated across the 8 gpsimd cores)."""
    n = len(flat)
    s = n // 16
    arr = np.empty((P, s), dtype=np.int16)
    blk = flat.reshape(s, 16).T  # [16, s]
    for grp in range(8):
        arr[grp * 16:(grp + 1) * 16, :] = blk
    return arr


def prepare(h, W, b, edges):
    h = np.asarray(h, dtype=np.float32)
    W = np.asarray(W, dtype=np.float32)
    b = np.asarray(b, dtype=np.float32)
    src = np.asarray(edges[0], dtype=np.int64)
    dst = np.asarray(edges[1], dtype=np.int64)

    out_deg = np.bincount(src, minlength=N_NODES).astype(np.float32)
    in_deg = np.bincount(dst, minlength=N_NODES).astype(np.float32)

    # global m-table row for each src node (padded per-core layout), then
    # paired-row coordinates: pair index + parity -> (sub-table, class)
    score = src // G
    mrow = score * GPAD + (src - score * G)
    pair = mrow >> 1
    par = mrow & 1
    qtab = pair // TROWS
    lrow = (pair - qtab * TROWS).astype(np.int16)
    cls = qtab * 2 + par

    dcore = dst // G
    dloc = dst - dcore * G
    grp = dloc // P
    ldst_v = (dloc - grp * P).astype(np.float32)

    # bucket = (dst-core, group, class)
    bucket = (dcore * NG + grp) * NCLS + cls
    order = np.argsort(bucket, kind="stable")
    bucket_s = bucket[order]
    lrow_s = lrow[order]
    ldst_s = ldst_v[order]

    nbuck = NCORES * NG * NCLS
    counts = np.bincount(bucket_s, minlength=nbuck).reshape(NCORES, NG, NCLS)
    starts = np.zeros(nbuck + 1, dtype=np.int64)
    np.cumsum(counts.reshape(-1), out=starts[1:])

    # uniform capacity per (group, class): max over cores, ceil to 128
    cap = counts.max(axis=0)                      # [NG, NCLS]
    cap128 = ((cap + P - 1) // P) * P             # [NG, NCLS]

    # round structure (uniform across cores)
    # gather tile columns: (class, g, chunk) order; ldst: (g, class, chunk)
    rounds = []
    idx_off = 0
    ch_off = 0
    r0 = 0
    for rsz in ROUND_SIZES:
        gs = list(range(r0, r0 + rsz))
        r0 += rsz
        q_numidx, q_choff = [], []
        gt_col = {}          # (g, c) -> gather-tile column base (within round)
        cursor = 0
        for c in range(NCLS):
            q_choff.append(cursor)
            tot = 0
            for g in gs:
                cp = int(cap128[g, c])
                gt_col[(g, c)] = cursor
                cursor += cp // P
                tot += cp
            q_numidx.append(tot)
        gldt, ggt = {}, {}
        lcur = 0
        for g in gs:
            cols = []
            for c in range(NCLS):
                cols.extend(range(gt_col[(g, c)],
                                  gt_col[(g, c)] + int(cap128[g, c]) // P))
            gldt[g] = (lcur, len(cols))
            ggt[g] = cols
            lcur += len(cols)
        rounds.append(dict(groups=gs, q_numidx=q_numidx, q_choff=q_choff,
                           nch=cursor, idx_off=idx_off, ch_off=ch_off,
                           gldt=gldt, ggt=ggt, gt_col=gt_col))
        idx_off += sum(q_numidx) // 16
        ch_off += cursor
    max_gch = max(rnd["gldt"][g][1] for rnd in rounds for g in rnd["groups"])
    meta = dict(rounds=rounds, tot_idx_cols=idx_off, tot_chunks=ch_off,
                max_gch=max_gch)

    # per-core gidx / ldst arrays
    gidx_cores = []
    ldst_cores = []
    for c0 in range(NCORES):
        flat_idx = np.zeros(idx_off * 16, dtype=np.int16)
        ld = np.full((P, ch_off), PAD_LDST, dtype=np.float32)
        for rnd in rounds:
            pos = rnd["idx_off"] * 16
            for c in range(NCLS):
                for g in rnd["groups"]:
                    bid = (c0 * NG + g) * NCLS + c
                    s0, s1 = starts[bid], starts[bid + 1]
                    n = s1 - s0
                    capq = int(cap128[g, c])
                    flat_idx[pos:pos + n] = lrow_s[s0:s1]
                    pos += capq
                    # ldst column base: group-major layout
                    qch0 = sum(int(cap128[g, c2]) // P for c2 in range(c))
                    base = rnd["ch_off"] + rnd["gldt"][g][0] + qch0
                    j = np.arange(n)
                    ld[j % P, base + j // P] = ldst_s[s0:s1]
        gidx_cores.append(_wrap_idx16(flat_idx))
        ldst_cores.append(ld.astype(ml_dtypes.bfloat16))

    # degree tiles [128, NG] (partition = node % 128 within group)
    def deg_tile(deg):
        tiles = []
        for c in range(NCORES):
            d = np.ones(GPAD, dtype=np.float32)
            d[:G] = deg[c * G:(c + 1) * G]
            tiles.append(d.reshape(NG, P).T.copy())
        return tiles

    odeg_tiles = deg_tile(out_deg)
    ideg_tiles = deg_tile(in_deg)

    hT_cores = []
    for c in range(NCORES):
        hp = np.zeros((GPAD, IN_DIM), dtype=np.float32)
        hp[:G] = h[c * G:(c + 1) * G]
        # [2, 128, GPAD]: k-halves, contiguous along nodes for wide DMAs
        ht = np.ascontiguousarray(hp.T.reshape(2, P, GPAD))
        hT_cores.append(ht.astype(ml_dtypes.bfloat16))

    wt = np.ascontiguousarray(W.reshape(2, P, OUT_DIM)).astype(
        ml_dtypes.bfloat16)
    brep = np.broadcast_to(b, (P, OUT_DIM)).copy()
    # iotar[p, m, c] = m
    iotar = np.broadcast_to(
        np.arange(P, dtype=np.float32)[None, :, None],
        (P, P, max_gch)).astype(ml_dtypes.bfloat16).copy()

    return dict(meta=meta, gidx=gidx_cores, ldst=ldst_cores,
                odeg=odeg_tiles, ideg=ideg_tiles, hT=hT_cores,
                W=wt, brep=brep, iotar=iotar)


_cache = {}


def _get_programs(meta):
    if "a" not in _cache:
        _cache["a"] = build_launch_a()
    if "b" not in _cache:
        _cache["b"] = build_launch_b(meta)
    return _cache["a"], _cache["b"]


def run_launch_a(nc_a, prep):
    in_maps = [{"hT": prep["hT"][c], "W": prep["W"], "odeg": prep["odeg"][c]}
               for c in range(NCORES)]
    res = run_bass_kernel_spmd(nc_a, in_maps, list(range(NCORES)))
    # m[p, g*64+f] -> rows (g*128+p, f)
    shards = []
    for r in res.results:
        md = np.asarray(r["m"]).reshape(P, NG, OUT_DIM)
        shards.append(md.transpose(1, 0, 2).reshape(GPAD, OUT_DIM))
    return shards


def run_launch_b(nc_b, prep, m_shards):
    m_full = np.concatenate(m_shards, axis=0)  # [NCORES*GPAD, 64] bf16
    mp = m_full.reshape(NPAIR, 2 * OUT_DIM)    # paired rows, 256 B stride
    tabs = {f"t{q}": np.ascontiguousarray(mp[q * TROWS:(q + 1) * TROWS])
            for q in range(NT)}
    in_maps = [dict(tabs, gidx=prep["gidx"][c], ldst=prep["ldst"][c],
                    ideg=prep["ideg"][c], brep=prep["brep"],
                    iotar=prep["iotar"]) for c in range(NCORES)]
    res = run_bass_kernel_spmd(nc_b, in_maps, list(range(NCORES)))
    outs = []
    for r in res.results:
        od = np.asarray(r["out"])  # [P, NG, 64]
        outs.append(od.transpose(1, 0, 2).reshape(GPAD, OUT_DIM)[:G])
    return np.concatenate(outs, axis=0)


def kernel(h, W, b, edges):
    prep = prepare(h, W, b, edges)
    nc_a, nc_b = _get_programs(prep["meta"])
    m_shards = run_launch_a(nc_a, prep)
    out = run_launch_b(nc_b, prep, m_shards)
    return out.astype(np.float32)


# revision 22
# speedup vs baseline: 2.2101x; 1.0216x over previous
"""GraphConv (DGL norm='both') + log_softmax on 8 Trainium2 NeuronCores.

Strategy (per sharding hint): partition nodes across the 8 cores by range.
  Launch A (per core): project its 12500-node slice m = (h @ W) * out_deg^-1/2
  in bf16 (PE bf16, PSUM f32 accumulate).
  Host: concatenate the 8 projected shards into a replicated gather table,
  viewed as PAIRED rows [50176, 128] bf16 so the table row stride is 256 B
  (DMA descriptor encoding granularity) while each gather moves only the
  needed 128-B half-row (the pair parity selects a 64-col offset).
  Launch B (per core): for its 12500 dst nodes, gather m[src] half-rows for
  all in-edges (dma_gather, edges pre-sorted by dst group), segment-sum via
  one-hot matmuls accumulating in PSUM, then norm/bias/log_softmax.

Degrees and the sorted/padded edge metadata are sharding-prep computed on the
host (numpy); all FLOPs on h/W/b/m (projection, normalization, aggregation,
softmax) run on device.
"""

import contextlib

import numpy as np
import ml_dtypes

import concourse.bass as bass
import concourse.bacc as bacc
import concourse.mybir as mybir
import concourse.tile as tile
from concourse.bass import AP
from concourse.bass_utils import run_bass_kernel_spmd

P = 128
N_NODES = 100000
N_EDGES = 3200000
IN_DIM = 256
OUT_DIM = 64
NCORES = 8
G = N_NODES // NCORES            # 12500 nodes per core
NG = (G + P - 1) // P            # 98 groups of 128 dst nodes (last has 84)
GPAD = NG * P                    # 12544
NPAIR = (NCORES * GPAD) // 2     # 50176 paired table rows
NT = 2                           # sub-tables (int16 index limit)
TROWS = NPAIR // NT              # 25088 rows per sub-table
NCLS = NT * 2                    # gather classes: (sub-table, parity)
ROUND_G = 8                      # max dst groups per gather round
# tapered round sizes: short first round starts compute early; short last
# rounds keep the post-gather drain chain small
ROUND_SIZES = [4] + [8] * 11 + [3, 2, 1]
assert sum(ROUND_SIZES) == NG
FIN_EVERY = 4                    # rounds per log_softmax finalize batch
HBLK = 16                        # dst groups per hT load in launch A
PAD_LDST = 200.0                 # local-dst for padded edges (>127, exact bf16)

_f32 = mybir.dt.float32
_bf16 = mybir.dt.bfloat16
_i16 = mybir.dt.int16


def _expand_mid(ap, n):
    """[P, C] AP -> [P, n, C] AP repeating each partition row n times
    (middle broadcast keeps the last dim packed, so DVE 2x mode applies)."""
    (ps, pc), (cs, cc) = ap.ap[0], ap.ap[1]
    return AP(ap.tensor, ap.offset, [[ps, pc], [0, n], [cs, cc]])


def _expand_last(ap, n):
    """[P, C] AP -> [P, C, n] AP repeating each element n times along a new
    innermost (stride-0) dim."""
    (ps, pc), (cs, cc) = ap.ap[0], ap.ap[1]
    return AP(ap.tensor, ap.offset, [[ps, pc], [cs, cc], [0, n]])


def _dma_gather_half(eng, out_ap, in_ap, idxs_ap, num_idxs, elem_size,
                     elem_step, single_packet=False):
    """dma_gather with a sub-256B payload (row stride must stay 256B-aligned:
    elem_step * dtype_size % 256 == 0). Same IR as bass's dma_gather helper,
    minus its payload-granularity assert (the HW descriptor only constrains
    the stride; the payload is free-form)."""
    stride_bytes = elem_step * mybir.dt.size(in_ap.dtype)
    assert stride_bytes % 256 == 0
    assert in_ap.ap[0][0] == elem_step
    return eng.add_instruction(
        mybir.InstDMAGatherAnt(
            name=eng.bass.get_next_instruction_name(),
            ins=[*eng.lower_ap_dma(in_ap, for_custom_bir_dma=True),
                 eng.lower_ap(idxs_ap),
                 eng.lower_val_access(eng.to_reg(num_idxs))],
            outs=[eng.lower_ap(out_ap)],
            transpose=False,
            num_idxs=num_idxs,
            elem_size=elem_size,
            stride_bytes_256=stride_bytes // 256,
            gen_mode=0,
            single_packet=single_packet,
            queue_num=0,
            sbuf_tokens_per_rank=0,
            sbuf_free_dim_per_rank=0,
            sbuf_free_dim_pad_per_rank=0,
            sbuf_byte_offset=0,
        ))


# ---------------------------------------------------------------- launch A
def build_launch_a(repeat=1):
    nc = bacc.Bacc("TRN2", target_bir_lowering=False, debug=False,
                   num_devices=NCORES)
    hT = nc.dram_tensor("hT", [2, P, GPAD], _bf16, kind="ExternalInput")
    W = nc.dram_tensor("W", [2, P, OUT_DIM], _bf16, kind="ExternalInput")
    odeg = nc.dram_tensor("odeg", [P, NG], _f32, kind="ExternalInput")
    # partition-major projected features: m[p, g*64+f] = m_row(g*128+p, f)
    m = nc.dram_tensor("m", [P, NG * OUT_DIM], _bf16, kind="ExternalOutput")

    with tile.TileContext(nc) as tc:
        loop = tc.For_i(0, repeat, 1) if repeat > 1 \
            else contextlib.nullcontext()
        with loop, \
                tc.tile_pool(name="const", bufs=1) as cpool, \
                tc.tile_pool(name="hblk", bufs=3) as hpool, \
                tc.tile_pool(name="mstage", bufs=2) as mpool, \
                tc.tile_pool(name="psum", bufs=8, space="PSUM") as psum:
            w0 = cpool.tile([P, OUT_DIM], _bf16, tag="w0")
            w1 = cpool.tile([P, OUT_DIM], _bf16, tag="w1")
            nc.sync.dma_start(out=w0[:], in_=W[0, :, :])
            nc.sync.dma_start(out=w1[:], in_=W[1, :, :])

            dt_ = cpool.tile([P, NG], _f32, tag="deg")
            norm = cpool.tile([P, NG], _f32, tag="norm")
            nc.sync.dma_start(out=dt_[:], in_=odeg[:, :])
            nc.vector.tensor_scalar_max(out=dt_[:], in0=dt_[:], scalar1=1.0)
            nc.vector.reciprocal(out=dt_[:], in_=dt_[:])
            nc.scalar.sqrt(out=norm[:], in_=dt_[:])

            for g0 in range(0, NG, HBLK):
                nb = min(HBLK, NG - g0)
                # both k-halves in one DMA: dram-side AP iterates (p, half,
                # node) to match the SBUF tile's (partition, half, node)
                lh = hpool.tile([P, 2, HBLK * P], _bf16, tag="lh")
                src = AP(hT[0, :, :].tensor, g0 * P,
                         [[GPAD, P], [P * GPAD, 2], [1, nb * P]])
                nc.sync.dma_start(out=lh[:, :, :nb * P], in_=src)
                ms = mpool.tile([P, HBLK, OUT_DIM], _bf16, tag="ms")
                # 8 groups per PSUM bank; norm scaling batched on DVE
                for j0 in range(0, nb, 8):
                    nj = min(8, nb - j0)
                    acc8 = psum.tile([P, 8, OUT_DIM], _f32, tag="acc8")
                    for j in range(j0, j0 + nj):
                        nc.tensor.matmul(acc8[:, j - j0, :],
                                         lh[:, 0, j * P:(j + 1) * P], w0[:],
                                         start=True, stop=False)
                        nc.tensor.matmul(acc8[:, j - j0, :],
                                         lh[:, 1, j * P:(j + 1) * P], w1[:],
                                         start=False, stop=True)
                    nc.vector.tensor_tensor(
                        out=ms[:, j0:j0 + nj, :], in0=acc8[:, :nj, :],
                        in1=_expand_last(norm[:, g0 + j0:g0 + j0 + nj],
                                         OUT_DIM),
                        op=mybir.AluOpType.mult)
                nc.gpsimd.dma_start(
                    out=m[:, g0 * OUT_DIM:(g0 + nb) * OUT_DIM],
                    in_=ms[:, :nb, :])
    nc.compile()
    return nc


# ---------------------------------------------------------------- launch B
def build_launch_b(meta, repeat=1):
    """meta["rounds"][i]:
      groups; q_numidx[NCLS]; q_choff[NCLS]; nch; idx_off; ch_off
      gldt: {g: (ldt_col_start, ngch)}   # ldst cols, group-major contiguous
      ggt:  {g: [gt_column, ...]}        # gather-tile column per oh chunk
    """
    nc = bacc.Bacc("TRN2", target_bir_lowering=False, debug=False,
                   num_devices=NCORES)
    tabs = [nc.dram_tensor(f"t{q}", [TROWS, 2 * OUT_DIM], _bf16,
                           kind="ExternalInput") for q in range(NT)]
    gidx = nc.dram_tensor("gidx", [P, meta["tot_idx_cols"]], _i16,
                          kind="ExternalInput")
    ldst = nc.dram_tensor("ldst", [P, meta["tot_chunks"]], _bf16,
                          kind="ExternalInput")
    max_gch = meta["max_gch"]
    ideg = nc.dram_tensor("ideg", [P, NG], _f32, kind="ExternalInput")
    brep = nc.dram_tensor("brep", [P, OUT_DIM], _f32, kind="ExternalInput")
    # iotar[p, m, c] = m  (bf16) — constant compare target for one-hot builds
    iotar = nc.dram_tensor("iotar", [P, P, max_gch], _bf16,
                           kind="ExternalInput")
    # partition-major output: out[p, g, f] = result(g*128+p, f)
    out = nc.dram_tensor("out", [P, NG, OUT_DIM], _f32, kind="ExternalOutput")

    with tile.TileContext(nc) as tc:
        loop = tc.For_i(0, repeat, 1) if repeat > 1 \
            else contextlib.nullcontext()
        with loop, \
                tc.tile_pool(name="const", bufs=1) as cpool, \
                tc.tile_pool(name="gath", bufs=2) as gpool, \
                tc.tile_pool(name="meta", bufs=4) as mpool, \
                tc.tile_pool(name="onehot", bufs=4) as opool, \
                tc.tile_pool(name="epi", bufs=3) as epool, \
                tc.tile_pool(name="yr", bufs=FIN_EVERY + 2) as ypool, \
                tc.tile_pool(name="psum", bufs=8, space="PSUM") as psum:
            bt = cpool.tile([P, OUT_DIM], _f32, tag="b")
            it = cpool.tile([P, P, max_gch], _bf16, tag="iotar")
            nc.sync.dma_start(out=bt[:], in_=brep[:, :])

            dt_ = cpool.tile([P, NG], _f32, tag="deg")
            norm = cpool.tile([P, NG], _f32, tag="norm")
            nc.sync.dma_start(out=dt_[:], in_=ideg[:, :])
            nc.vector.tensor_scalar_max(out=dt_[:], in0=dt_[:], scalar1=1.0)
            nc.vector.reciprocal(out=dt_[:], in_=dt_[:])
            nc.scalar.sqrt(out=norm[:], in_=dt_[:])

            s_all = cpool.tile([P, NG], _f32, tag="sall")
            ls_all = cpool.tile([P, NG], _f32, tag="lsall")

            yr_tiles = {}
            for ri, rnd in enumerate(meta["rounds"]):
                gs = rnd["groups"]
                rg = len(gs)
                nch = rnd["nch"]
                nidx_cols = sum(rnd["q_numidx"]) // 16
                ixt = mpool.tile([P, nidx_cols], _i16, tag="ix")
                nc.sync.dma_start(
                    out=ixt[:],
                    in_=gidx[:, rnd["idx_off"]:rnd["idx_off"] + nidx_cols])
                ldt = mpool.tile([P, nch], _bf16, tag="ld")
                nc.scalar.dma_start(
                    out=ldt[:],
                    in_=ldst[:, rnd["ch_off"]:rnd["ch_off"] + nch])

                gt = gpool.tile([P, nch, OUT_DIM], _bf16, tag="gt")
                icol = 0
                for q in range(NT):
                    for par in range(2):
                        nq = rnd["q_numidx"][q * 2 + par]
                        if nq == 0:
                            continue
                        co = rnd["q_choff"][q * 2 + par]
                        _dma_gather_half(
                            nc.gpsimd,
                            out_ap=gt[:, co:co + nq // P, :],
                            in_ap=tabs[q][:, par * OUT_DIM:(par + 1) * OUT_DIM],
                            idxs_ap=ixt[:, icol:icol + nq // 16],
                            num_idxs=nq,
                            elem_size=OUT_DIM,
                            elem_step=2 * OUT_DIM,
                            single_packet=False,
                        )
                        icol += nq // 16
                if ri == 0:
                    # issued after round 0's gathers so it doesn't delay them
                    nc.scalar.dma_start(out=it[:], in_=iotar[:, :, :])

                xr = epool.tile([P, ROUND_G, OUT_DIM], _f32, tag="xr")
                for i, g in enumerate(gs):
                    ldt0, ngch = rnd["gldt"][g]
                    gtcols = rnd["ggt"][g]
                    # one-hot, chunk-last: oh[k, m, c] = (ldst[k,col_c]==m)
                    # all APs keep a packed last dim -> DVE 2x mode
                    oh = opool.tile([P, P, max_gch], _bf16, tag="oh")
                    nc.vector.tensor_tensor(
                        out=oh[:, :, 0:ngch],
                        in0=_expand_mid(ldt[:, ldt0:ldt0 + ngch], P),
                        in1=it[:, :, 0:ngch],
                        op=mybir.AluOpType.is_equal)
                    acc = psum.tile([P, OUT_DIM], _f32, tag="acc")
                    for k, cg in enumerate(gtcols):
                        nc.tensor.matmul(
                            acc[:], oh[:, :, k], gt[:, cg, :],
                            start=(k == 0), stop=(k == ngch - 1))
                    nc.scalar.activation(
                        out=xr[:, i, :], in_=acc[:],
                        func=mybir.ActivationFunctionType.Identity,
                        scale=norm[:, g:g + 1])

                g0 = gs[0]
                # batched epilogue for the round's rg groups
                nc.vector.tensor_tensor(
                    out=xr[:, :rg, :], in0=xr[:, :rg, :],
                    in1=_expand_mid(bt[:, :], rg),
                    op=mybir.AluOpType.add)
                nmx = epool.tile([P, ROUND_G], _f32, tag="nmx")
                nc.vector.tensor_reduce(out=nmx[:, :rg], in_=xr[:, :rg, :],
                                        axis=mybir.AxisListType.X,
                                        op=mybir.AluOpType.max,
                                        negate=True)
                yr = ypool.tile([P, ROUND_G, OUT_DIM], _f32, tag="yr")
                yr_tiles[ri] = yr
                nc.vector.tensor_tensor(
                    out=yr[:, :rg, :], in0=xr[:, :rg, :],
                    in1=_expand_last(nmx[:, :rg], OUT_DIM),
                    op=mybir.AluOpType.add)
                e = epool.tile([P, ROUND_G, OUT_DIM], _f32, tag="e")
                for i, g in enumerate(gs):
                    nc.scalar.activation(
                        out=e[:, i, :], in_=yr[:, i, :],
                        func=mybir.ActivationFunctionType.Exp,
                        accum_out=s_all[:, g:g + 1])
                # clustered log_softmax finalize: one Ln per FIN_EVERY rounds
                # keeps Exp<->Ln act-table swaps off the per-round path
                if (ri + 1) % FIN_EVERY == 0 or ri == len(meta["rounds"]) - 1:
                    r_lo = (ri // FIN_EVERY) * FIN_EVERY
                    glo = meta["rounds"][r_lo]["groups"][0]
                    nc.scalar.activation(
                        out=ls_all[:, glo:g0 + rg],
                        in_=s_all[:, glo:g0 + rg],
                        func=mybir.ActivationFunctionType.Ln)
                    for rj in range(r_lo, ri + 1):
                        gsj = meta["rounds"][rj]["groups"]
                        gj0, rgj = gsj[0], len(gsj)
                        yj = yr_tiles.pop(rj)
                        nc.vector.tensor_tensor(
                            out=yj[:, :rgj, :], in0=yj[:, :rgj, :],
                            in1=_expand_last(ls_all[:, gj0:gj0 + rgj],
                                             OUT_DIM),
                            op=mybir.AluOpType.subtract)
                        nc.scalar.dma_start(out=out[:, gj0:gj0 + rgj, :],
                                            in_=yj[:, :rgj, :])
    nc.compile()
    return nc


# ------------------------------------------------------------- host prep
def _wrap_idx16(flat):
    """int16 index list -> [128, len/16] wrapped layout (16-partition groups,
    replicated across the 8 gpsimd cores)."""
    n = len(flat)
    s = n // 16
    arr = np.empty((P, s), dtype=np.int16)
    blk = flat.reshape(s, 16).T  # [16, s]
    for grp in range(8):
        arr[grp * 16:(grp + 1) * 16, :] = blk
    return arr


def prepare(h, W, b, edges):
    h = np.asarray(h, dtype=np.float32)
    W = np.asarray(W, dtype=np.float32)
    b = np.asarray(b, dtype=np.float32)
    src = np.asarray(edges[0], dtype=np.int64)
    dst = np.asarray(edges[1], dtype=np.int64)

    out_deg = np.bincount(src, minlength=N_NODES).astype(np.float32)
    in_deg = np.bincount(dst, minlength=N_NODES).astype(np.float32)

    # global m-table row for each src node (padded per-core layout), then
    # paired-row coordinates: pair index + parity -> (sub-table, class)
    score = src // G
    mrow = score * GPAD + (src - score * G)
    pair = mrow >> 1
    par = mrow & 1
    qtab = pair // TROWS
    lrow = (pair - qtab * TROWS).astype(np.int16)
    cls = qtab * 2 + par

    dcore = dst // G
    dloc = dst - dcore * G
    grp = dloc // P
    ldst_v = (dloc - grp * P).astype(np.float32)

    # bucket = (dst-core, group, class)
    bucket = (dcore * NG + grp) * NCLS + cls
    order = np.argsort(bucket, kind="stable")
    bucket_s = bucket[order]
    lrow_s = lrow[order]
    ldst_s = ldst_v[order]

    nbuck = NCORES * NG * NCLS
    counts = np.bincount(bucket_s, minlength=nbuck).reshape(NCORES, NG, NCLS)
    starts = np.zeros(nbuck + 1, dtype=np.int64)
    np.cumsum(counts.reshape(-1), out=starts[1:])

    # uniform capacity per (group, class): max over cores, ceil to 128
    cap = counts.max(axis=0)                      # [NG, NCLS]
    cap128 = ((cap + P - 1) // P) * P             # [NG, NCLS]

    # round structure (uniform across cores)
    # gather tile columns: (class, g, chunk) order; ldst: (g, class, chunk)
    rounds = []
    idx_off = 0
    ch_off = 0
    r0 = 0
    for rsz in ROUND_SIZES:
        gs = list(range(r0, r0 + rsz))
        r0 += rsz
        q_numidx, q_choff = [], []
        gt_col = {}          # (g, c) -> gather-tile column base (within round)
        cursor = 0
        for c in range(NCLS):
            q_choff.append(cursor)
            tot = 0
            for g in gs:
                cp = int(cap128[g, c])
                gt_col[(g, c)] = cursor
                cursor += cp // P
                tot += cp
            q_numidx.append(tot)
        gldt, ggt = {}, {}
        lcur = 0
        for g in gs:
            cols = []
            for c in range(NCLS):
                cols.extend(range(gt_col[(g, c)],
                                  gt_col[(g, c)] + int(cap128[g, c]) // P))
            gldt[g] = (lcur, len(cols))
            ggt[g] = cols
            lcur += len(cols)
        rounds.append(dict(groups=gs, q_numidx=q_numidx, q_choff=q_choff,
                           nch=cursor, idx_off=idx_off, ch_off=ch_off,
                           gldt=gldt, ggt=ggt, gt_col=gt_col))
        idx_off += sum(q_numidx) // 16
        ch_off += cursor
    max_gch = max(rnd["gldt"][g][1] for rnd in rounds for g in rnd["groups"])
    meta = dict(rounds=rounds, tot_idx_cols=idx_off, tot_chunks=ch_off,
                max_gch=max_gch)

    # per-core gidx / ldst arrays
    gidx_cores = []
    ldst_cores = []
    for c0 in range(NCORES):
        flat_idx = np.zeros(idx_off * 16, dtype=np.int16)
        ld = np.full((P, ch_off), PAD_LDST, dtype=np.float32)
        for rnd in rounds:
            pos = rnd["idx_off"] * 16
            for c in range(NCLS):
                for g in rnd["groups"]:
                    bid = (c0 * NG + g) * NCLS + c
                    s0, s1 = starts[bid], starts[bid + 1]
                    n = s1 - s0
                    capq = int(cap128[g, c])
                    flat_idx[pos:pos + n] = lrow_s[s0:s1]
                    pos += capq
                    # ldst column base: group-major layout
                    qch0 = sum(int(cap128[g, c2]) // P for c2 in range(c))
                    base = rnd["ch_off"] + rnd["gldt"][g][0] + qch0
                    j = np.arange(n)
                    ld[j % P, base + j // P] = ldst_s[s0:s1]
        gidx_cores.append(_wrap_idx16(flat_idx))
        ldst_cores.append(ld.astype(ml_dtypes.bfloat16))

    # degree tiles [128, NG] (partition = node % 128 within group)
    def deg_tile(deg):
        tiles = []
        for c in range(NCORES):
            d = np.ones(GPAD, dtype=np.float32)
            d[:G] = deg[c * G:(c + 1) * G]
            tiles.append(d.reshape(NG, P).T.copy())
        return tiles

    odeg_tiles = deg_tile(out_deg)
    ideg_tiles = deg_tile(in_deg)

    hT_cores = []
    for c in range(NCORES):
        hp = np.zeros((GPAD, IN_DIM), dtype=np.float32)
        hp[:G] = h[c * G:(c + 1) * G]
        # [2, 128, GPAD]: k-halves, contiguous along nodes for wide DMAs
        ht = np.ascontiguousarray(hp.T.reshape(2, P, GPAD))
        hT_cores.append(ht.astype(ml_dtypes.bfloat16))

    wt = np.ascontiguousarray(W.reshape(2, P, OUT_DIM)).astype(
        ml_dtypes.bfloat16)
    brep = np.broadcast_to(b, (P, OUT_DIM)).copy()
    # iotar[p, m, c] = m
    iotar = np.broadcast_to(
        np.arange(P, dtype=np.float32)[None, :, None],
        (P, P, max_gch)).astype(ml_dtypes.bfloat16).copy()

    return dict(meta=meta, gidx=gidx_cores, ldst=ldst_cores,
                odeg=odeg_tiles, ideg=ideg_tiles, hT=hT_cores,
                W=wt, brep=brep, iotar=iotar)


_cache = {}


def _get_programs(meta):
    if "a" not in _cache:
        _cache["a"] = build_launch_a()
    if "b" not in _cache:
        _cache["b"] = build_launch_b(meta)
    return _cache["a"], _cache["b"]


def run_launch_a(nc_a, prep):
    in_maps = [{"hT": prep["hT"][c], "W": prep["W"], "odeg": prep["odeg"][c]}
               for c in range(NCORES)]
    res = run_bass_kernel_spmd(nc_a, in_maps, list(range(NCORES)))
    # m[p, g*64+f] -> rows (g*128+p, f)
    shards = []
    for r in res.results:
        md = np.asarray(r["m"]).reshape(P, NG, OUT_DIM)
        shards.append(md.transpose(1, 0, 2).reshape(GPAD, OUT_DIM))
    return shards


def run_launch_b(nc_b, prep, m_shards):
    m_full = np.concatenate(m_shards, axis=0)  # [NCORES*GPAD, 64] bf16
    mp = m_full.reshape(NPAIR, 2 * OUT_DIM)    # paired rows, 256 B stride
    tabs = {f"t{q}": np.ascontiguousarray(mp[q * TROWS:(q + 1) * TROWS])
            for q in range(NT)}
    in_maps = [dict(tabs, gidx=prep["gidx"][c], ldst=prep["ldst"][c],
                    ideg=prep["ideg"][c], brep=prep["brep"],
                    iotar=prep["iotar"]) for c in range(NCORES)]
    res = run_bass_kernel_spmd(nc_b, in_maps, list(range(NCORES)))
    outs = []
    for r in res.results:
        od = np.asarray(r["out"])  # [P, NG, 64]
        outs.append(od.transpose(1, 0, 2).reshape(GPAD, OUT_DIM)[:G])
    return np.concatenate(outs, axis=0)


def kernel(h, W, b, edges):
    prep = prepare(h, W, b, edges)
    nc_a, nc_b = _get_programs(prep["meta"])
    m_shards = run_launch_a(nc_a, prep)
    out = run_launch_b(nc_b, prep, m_shards)
    return out.astype(np.float32)
